# revision 1
# baseline (speedup 1.0000x reference)
"""BlockGlobalAttentionProduct Trainium2 kernel.

Sharding: 24 (n,h) pairs across 8 cores, 3 per core. Each core, per (n,h):
  - dma_gather of interleaved [K|V] bf16 rows (256B) by local_idx / global_idx
  - PE transposes build K^T (d on partitions) for the score matmuls
  - scores^T computed per key tile (keys on partitions, queries on free dim)
  - exp on ScalarE (scale=1/8 folded in); window padding masked by zeroing
  - PV accumulated in ctx^T form (d+1 rows incl. sum-of-exp) in PSUM
  - host does final divide-by-denominator + transpose during unshard
"""

import sys

sys.path.insert(0, "/opt/trn_rl_repo")

import numpy as np
import ml_dtypes

import concourse.bacc as bacc
import concourse.mybir as mybir
from concourse import bass, tile, bass_utils, library_config

# problem shape (hardcoded per spec)
N, H, T, D = 2, 12, 4096, 64
NH = N * H            # 24
NCORES = 8
PER_CORE = NH // NCORES  # 3
NTILE = T // 128      # 32 key tiles per table
NSEG = 8              # query segments of 512
QH_W = 128 + T + 256  # qT halo width: cols [-128, 4352)
NEG0 = 0

BF16 = mybir.dt.bfloat16
F32 = mybir.dt.float32
I16 = mybir.dt.int16


def _intervals(a0, width, s):
    """Pieces of window [a0, a0+width) mod T intersected with segment
    [512s, 512(s+1)). Yields (tile_col_offset, seg_col_offset, length)."""
    lo, hi = 512 * s, 512 * (s + 1)
    pieces = []
    a0 %= T
    if a0 + width <= T:
        pieces.append((a0, a0 + width, 0))
    else:
        pieces.append((a0, T, 0))
        pieces.append((0, (a0 + width) % T, T - a0))
    out = []
    for wa, wb, base in pieces:
        u, v = max(wa, lo), min(wb, hi)
        if u < v:
            out.append((base + (u - wa), u - lo, v - u))
    return out


def build_program():
    nc = bacc.Bacc("TRN2", target_bir_lowering=False, debug=False,
                   num_devices=NCORES)

    qTh = nc.dram_tensor("qTh", [PER_CORE, 64, QH_W], BF16, kind="ExternalInput")
    kvT = nc.dram_tensor("kv", [PER_CORE, T, 128], BF16, kind="ExternalInput")
    gkT_d = nc.dram_tensor("gkT", [PER_CORE, 64, 64], BF16, kind="ExternalInput")
    # gv1[:, :, p, :]: [gv|1] rows zero-padded on the opposite 64-partition
    # half, so gtok PV can contract the full 128 partitions of the
    # column-paired expT layout (parity p selects which half is live).
    gv1_d = nc.dram_tensor("gv1", [PER_CORE, 128, 2, 65], BF16, kind="ExternalInput")
    lidx_d = nc.dram_tensor("lidx", [PER_CORE, 128, 256], I16, kind="ExternalInput")
    gidx_d = nc.dram_tensor("gidx", [PER_CORE, 128, 256], I16, kind="ExternalInput")
    ident_d = nc.dram_tensor("ident", [128, 128], BF16, kind="ExternalInput")
    out_d = nc.dram_tensor("ctxT", [PER_CORE, 65, T], F32, kind="ExternalOutput")

    EXP = mybir.ActivationFunctionType.Exp

    with tile.TileContext(nc) as tc:
        with (
            tc.tile_pool(name="const", bufs=1) as constp,
            tc.tile_pool(name="land", bufs=2) as land,
            tc.tile_pool(name="work", bufs=1) as work,
            tc.tile_pool(name="outp", bufs=2) as outp,
            tc.tile_pool(name="ps1", bufs=2, space="PSUM") as ps1,
            tc.tile_pool(name="psL", bufs=1, space="PSUM") as psL,
            tc.tile_pool(name="psG", bufs=1, space="PSUM") as psG,
        ):
            ident = constp.tile([128, 128], BF16, tag="ident")
            nc.sync.dma_start(ident[:], ident_d[:])
            lib_i = nc.gpsimd.load_library(library_config.mlp)

            for i in range(PER_CORE):
                # ---------------- loads + gathers ----------------
                q_sb = land.tile([64, QH_W], BF16, tag="q")
                kvL = land.tile([128, NTILE, 128], BF16, tag="kvL")
                kvG = land.tile([128, NTILE, 128], BF16, tag="kvG")
                li_sb = land.tile([128, 256], I16, tag="li")
                gi_sb = land.tile([128, 256], I16, tag="gi")
                gkT = land.tile([64, 64], BF16, tag="gkT")
                gv1 = land.tile([128, 2, 65], BF16, tag="gv1")

                nc.sync.dma_start(q_sb[:], qTh[i])
                nc.sync.dma_start(gkT[:], gkT_d[i])
                nc.sync.dma_start(gv1[:], gv1_d[i])
                nc.gpsimd.dma_start(li_sb[:], lidx_d[i])
                nc.gpsimd.dma_start(gi_sb[:], gidx_d[i])
                g1 = nc.gpsimd.dma_gather(kvL[:], kvT[i], li_sb[:], T, T, 128,
                                          single_packet=False)
                g2 = nc.gpsimd.dma_gather(kvG[:], kvT[i], gi_sb[:], T, T, 128,
                                          single_packet=False)
                if i == 0:
                    from concourse.tile_rust import add_dep_helper
                    add_dep_helper(lib_i.ins, g1.ins, reason="lib before gather")

                # ---------------- K^T construction ----------------
                # all K^T tiles live on partitions [0,64) — the PE on this
                # runtime rejects row-group (contraction base) alternation,
                # so every score matmul contracts at base partition 0.
                klT = work.tile([64, 4096], BF16, tag="klT")
                kgT = work.tile([64, 4096], BF16, tag="kgT")
                for kv_sb, kT in ((kvL, klT), (kvG, kgT)):
                    for grp in range(4):         # 8 tiles per psum pack
                        tp = ps1.tile([64, 1024], BF16, tag="b1")
                        for pp in range(8):
                            c = grp * 8 + pp
                            nc.tensor.transpose(
                                out=tp[:, pp * 128:(pp + 1) * 128],
                                in_=kv_sb[:, c, 0:64], identity=ident[:])
                        nc.vector.tensor_copy(
                            kT[:, grp * 1024:(grp + 1) * 1024], tp[:])

                # ---------------- V1 = [V | 1] ----------------
                v1L = work.tile([128, NTILE, 65], BF16, tag="v1L")
                v1G = work.tile([128, NTILE, 65], BF16, tag="v1G")
                for kv_sb, v1 in ((kvL, v1L), (kvG, v1G)):
                    nc.gpsimd.memset(v1[:, :, 64:65], 1.0)
                    nc.vector.tensor_copy(v1[:, :, 0:64], kv_sb[:, :, 64:128])

                # ---------------- scores^T + exp ----------------
                expL = work.tile([128, NTILE, 256], BF16, tag="expL")
                expG = work.tile([128, NTILE, 384], BF16, tag="expG")
                expT = work.tile([128, 4, 512], BF16, tag="expT")

                # local: per key tile c, queries [(2c-1)*64, (2c+3)*64)
                # two col-group matmuls per tile (key halves at output
                # partition halves) — contraction base 0 for both.
                for p in range(8):               # packs of 4 tiles
                    st = psL.tile([128, 1024], F32, tag="pL")
                    for j in range(4):
                        c = 4 * p + j
                        rhs = q_sb[:, 64 + 128 * c:64 + 128 * c + 256]
                        nc.tensor.matmul(st[:, j * 256:(j + 1) * 256],
                                         klT[:, 128 * c:128 * c + 128], rhs,
                                         start=True, stop=True)
                    nc.scalar.activation(expL[:, 4 * p:4 * p + 4, :],
                                         st[:].rearrange("p (a b) -> p a b", b=256),
                                         EXP, scale=0.125)
                    for j in range(4):
                        c = 4 * p + j
                        nc.gpsimd.memset(expL[64:128, c, 0:64], NEG0)
                        nc.gpsimd.memset(expL[0:64, c, 192:256], NEG0)

                # global: per key tile t, queries [(t-1)*128, (t+2)*128)
                for p in range(8):
                    st = psG.tile([128, 2048], F32, tag="pG")
                    for j in range(4):
                        t = 4 * p + j
                        rhs = q_sb[:, 128 * t:128 * t + 384]
                        nc.tensor.matmul(st[:, j * 512:j * 512 + 384],
                                         kgT[:, 128 * t:128 * t + 128], rhs,
                                         start=True, stop=True)
                    src = st[:].rearrange("p (a b) -> p a b", b=512)[:, :, 0:384]
                    nc.scalar.activation(expG[:, 4 * p:4 * p + 4, :], src,
                                         EXP, scale=0.125)

                # gtok: per query block g of 512
                for p in range(4):
                    st = ps1.tile([128, 512], F32, tag="b1")
                    for j in range(2):
                        g = 2 * p + j
                        nc.tensor.matmul(
                            st[j * 64:j * 64 + 64, 0:512],
                            gkT[:], q_sb[:, 128 + 512 * g:128 + 512 * g + 512],
                            start=True, stop=True,
                            tile_position=(0, j * 64))
                    nc.scalar.activation(expT[:, p, :], st[:], EXP, scale=0.125)

                # ---------------- PV (ctx^T accumulate) ----------------
                ctx_sb = outp.tile([65, T], F32, tag="ctx")
                for s in range(NSEG):
                    acc = ps1.tile([65, 512], F32, tag="b1")
                    mms = []
                    # gtok initializes the whole segment (full-128 contraction;
                    # the inactive parity half of gv1 is zero)
                    mms.append((gv1[:, s % 2, :], expT[:, s // 2, 0:512], 0, 512))
                    for c in range(NTILE):
                        for (tcol, scol, ln) in _intervals((2 * c - 1) * 64, 256, s):
                            mms.append((v1L[:, c, :],
                                        expL[:, c, tcol:tcol + ln], scol, ln))
                    for t in range(NTILE):
                        for (tcol, scol, ln) in _intervals((t - 1) * 128, 384, s):
                            mms.append((v1G[:, t, :],
                                        expG[:, t, tcol:tcol + ln], scol, ln))
                    for mi, (lhsT, rhs, scol, ln) in enumerate(mms):
                        nc.tensor.matmul(acc[:, scol:scol + ln], lhsT, rhs,
                                         start=(mi == 0), stop=(mi == len(mms) - 1),
                                         skip_group_check=True)
                    nc.vector.tensor_copy(ctx_sb[:, 512 * s:512 * (s + 1)], acc[:])

                nc.sync.dma_start(out_d[i], ctx_sb[:])

    nc.compile()
    return nc


_CACHED = None


def _get_program():
    global _CACHED
    if _CACHED is None:
        _CACHED = build_program()
    return _CACHED


def _prep_core_inputs(q, k, v, gk, gv, lidx, gidx, pairs):
    """Build one core's input dict for its list of (n,h) pairs."""
    bf = ml_dtypes.bfloat16
    qTh = np.empty((PER_CORE, 64, QH_W), dtype=bf)
    kv = np.empty((PER_CORE, T, 128), dtype=bf)
    gkT = np.empty((PER_CORE, 64, 64), dtype=bf)
    gv1 = np.zeros((PER_CORE, 128, 2, 65), dtype=bf)
    li = np.empty((PER_CORE, 128, 256), dtype=np.int16)
    gi = np.empty((PER_CORE, 128, 256), dtype=np.int16)
    for s, (n, h) in enumerate(pairs):
        qt = np.ascontiguousarray(q[n, h].T)            # (64, T) f32
        qth = np.concatenate([qt[:, T - 128:], qt, qt[:, :256]], axis=1)
        qTh[s] = qth.astype(bf)
        kv[s, :, 0:64] = k[n, h].astype(bf)
        kv[s, :, 64:128] = v[n, h].astype(bf)
        gkT[s] = np.ascontiguousarray(gk[n, h].T).astype(bf)
        g1 = np.concatenate([gv[n, h], np.ones((64, 1), np.float32)],
                            axis=1).astype(bf)
        gv1[s, 0:64, 0] = g1      # parity 0: top half live
        gv1[s, 64:128, 1] = g1    # parity 1: bottom half live
        for arr, src in ((li, lidx), (gi, gidx)):
            ix = src[n, h, :, 0].astype(np.int16)       # (T,)
            arr[s] = np.tile(ix.reshape(T // 16, 16).T, (8, 1))
    ident = np.eye(128, dtype=bf)
    return {"qTh": qTh, "kv": kv, "gkT": gkT, "gv1": gv1,
            "lidx": li, "gidx": gi, "ident": ident}


def kernel(query_layer, key_layer, value_layer, attention_mask, local_idx,
           global_idx, global_key, global_value, global_mask):
    # attention_mask / global_mask are all-zero in this problem's input spec;
    # they contribute nothing to the scores and are not shipped to the device.
    q = np.asarray(query_layer, np.float32)
    k = np.asarray(key_layer, np.float32)
    v = np.asarray(value_layer, np.float32)
    gk = np.asarray(global_key, np.float32)
    gv = np.asarray(global_value, np.float32)
    li = np.asarray(local_idx)
    gi = np.asarray(global_idx)

    nc = _get_program()
    in_maps = []
    for m in range(NCORES):
        pairs = [((3 * m + s) // H, (3 * m + s) % H) for s in range(PER_CORE)]
        in_maps.append(_prep_core_inputs(q, k, v, gk, gv, li, gi, pairs))
    res = bass_utils.run_bass_kernel_spmd(nc, in_maps, core_ids=list(range(NCORES)))

    out = np.empty((N, H, T, D), np.float32)
    for m in range(NCORES):
        ctxT = res.results[m]["ctxT"]                   # (3, 65, T)
        for s in range(PER_CORE):
            n, h = (3 * m + s) // H, (3 * m + s) % H
            out[n, h] = (ctxT[s, :64] / ctxT[s, 64:65]).T
    return out



# revision 3
# speedup vs baseline: 1.4337x; 1.4337x over previous
"""BlockGlobalAttentionProduct Trainium2 kernel (v2).

Sharding: 24 (n,h) pairs across 8 cores, 3 per core. Per (n,h):
  - kv table rows in DRAM: [K bf16 64 | V bf16 64], 256B/row.
  - transpose-mode dma_gather lands K^T (d on partitions 0:64) and V^T
    (partitions 64:128) directly in SBUF - no PE K-transposes, no PSUM->SBUF
    K copies. A [64,128] PE transpose per key tile turns V^T back into V
    rows ([V|1] with a memset ones column -> denominator in row 64).
  - QK in bf16 -> PSUM f32 -> ScalarE exp (scale 1/8) -> bf16 score tiles;
    local-window staircase corners zeroed by Pool memsets.
  - PV in bf16: per 512-query PSUM segment, one matmul per intersecting
    key-tile window (6 local + 6 global + 1 global-token piece).
  - host divides by the row-64 denominator + transposes during unshard.
"""

import sys

sys.path.insert(0, "/opt/trn_rl_repo")

import numpy as np
import ml_dtypes

import concourse.bacc as bacc
import concourse.mybir as mybir
from concourse import bass, tile, bass_utils, library_config

N, H, T, D = 2, 12, 4096, 64
NH = N * H
NCORES = 8
PER_CORE = NH // NCORES   # 3
NT = T // 128             # 32 key tiles per table
QH_W = 128 + T + 256      # q^T halo: cols [-128, 4352)

BF16 = mybir.dt.bfloat16
F32 = mybir.dt.float32
I16 = mybir.dt.int16
EXP = mybir.ActivationFunctionType.Exp
# bf16-Schraudolph: trunc(x*S16 + B16) as int16 is the bf16 bit pattern of
# ~e^(x/8) (max rel err ~3.5%); used on the DVE for part of the exp work
S16 = float(16.0 * np.log2(np.e))
B16 = float(128.0 * (127.0 - 0.0430))
# per-pack exp engine for the global table on pipelined pairs:
# v=DVE bit-trick, p=Pool bit-trick, a=ScalarE exact
GPACK_ENG = {0: "v", 1: "v", 2: "v", 3: "a", 4: "v", 5: "v", 6: "v", 7: "a"}
GPACK_ENG_LAST = {0: "v", 2: "v", 4: "v", 6: "v"}


def build_program():
    nc = bacc.Bacc("TRN2", target_bir_lowering=False, debug=False,
                   num_devices=NCORES)

    qTh = nc.dram_tensor("qTh", [PER_CORE, 64, QH_W], BF16, kind="ExternalInput")
    kvtab = nc.dram_tensor("kvtab", [PER_CORE, T, 128], BF16, kind="ExternalInput")
    gkT_d = nc.dram_tensor("gkT", [PER_CORE, 64, 64], BF16, kind="ExternalInput")
    gv1_d = nc.dram_tensor("gv1", [PER_CORE, 128, 2, 65], BF16, kind="ExternalInput")
    lidx_d = nc.dram_tensor("lidx", [PER_CORE, 128, 256], I16, kind="ExternalInput")
    gidx_d = nc.dram_tensor("gidx", [PER_CORE, 128, 256], I16, kind="ExternalInput")
    ident_d = nc.dram_tensor("ident", [128, 128], BF16, kind="ExternalInput")
    out_d = nc.dram_tensor("ctxT", [PER_CORE, 128, NT, 65], BF16, kind="ExternalOutput")

    with tile.TileContext(nc) as tc:
        with (
            tc.tile_pool(name="const", bufs=1) as constp,
            tc.tile_pool(name="land", bufs=2) as land,
            tc.tile_pool(name="ktp", bufs=2) as ktp,
            tc.tile_pool(name="v1p", bufs=2) as v1p,
            tc.tile_pool(name="expp", bufs=2) as expp,
            tc.tile_pool(name="outp", bufs=2) as outp,
            tc.tile_pool(name="psq", bufs=2, space="PSUM") as psq,
            tc.tile_pool(name="psa", bufs=2, space="PSUM") as psa,
        ):
            ident = constp.tile([128, 128], BF16, tag="ident")
            lib_i = nc.gpsimd.load_library(library_config.mlp)
            first_gather = [True]

            def loads(i):
                q_sb = land.tile([64, QH_W], BF16, tag="q")
                gkT = land.tile([64, 64], BF16, tag="gkT")
                gv1 = land.tile([128, 2, 65], BF16, tag="gv1")
                li = land.tile([128, 256], I16, tag="li")
                gi = land.tile([128, 256], I16, tag="gi")
                kTL = ktp.tile([128, T], BF16, tag="kTL")
                kTG = ktp.tile([128, T], BF16, tag="kTG")
                dma_q = nc.gpsimd if i == 0 else nc.sync
                dma_q.dma_start(li[:], lidx_d[i])
                dma_q.dma_start(gi[:], gidx_d[i])
                nc.sync.dma_start(q_sb[:], qTh[i])
                nc.sync.dma_start(gkT[:], gkT_d[i])
                nc.sync.dma_start(gv1[:], gv1_d[i])
                nh = 4 if i == 0 else 2      # pair 0: quarter-gathers
                for kT, idx in ((kTL, li), (kTG, gi)):
                    w = T // nh
                    for h in range(nh):
                        dst = kT[:, w * h:w * (h + 1)].rearrange(
                            "p (a b) -> p a b", a=1)
                        g = nc.gpsimd.dma_gather(
                            dst, kvtab[i],
                            idx[:, (w // 16) * h:(w // 16) * (h + 1)],
                            w, w, 128, transpose=True,
                            single_packet=False)
                        if first_gather[0]:
                            from concourse.tile_rust import add_dep_helper
                            add_dep_helper(lib_i.ins, g.ins,
                                           reason="lib before gather")
                            first_gather[0] = False
                return dict(q=q_sb, gkT=gkT, gv1=gv1, kTL=kTL, kTG=kTG)

            def vt_init(st_):
                v1L = v1p.tile([128, NT, 66], BF16, tag="v1L")
                v1G = v1p.tile([128, NT, 66], BF16, tag="v1G")
                nc.gpsimd.memset(v1L[:, :, 64:65], 1.0)
                nc.gpsimd.memset(v1G[:, :, 64:65], 1.0)
                st_["v1L"], st_["v1G"] = v1L, v1G

            def vt_pack(st_, j):
                # V rows: transpose V^T (partitions 64:128 of the gathered
                # tiles) back to keys-on-partitions, 8 key tiles per psum pack
                kT = st_["kTL"] if j < 4 else st_["kTG"]
                v1 = st_["v1L"] if j < 4 else st_["v1G"]
                p = j % 4
                tp = psq.tile([128, 512], BF16, tag="qk")
                for k in range(8):
                    c = 8 * p + k
                    nc.tensor.transpose(
                        tp[:, 64 * k:64 * k + 64],
                        in_=kT[64:128, 128 * c:128 * c + 128],
                        identity=ident[64:128, 64:128])
                nc.vector.tensor_copy(
                    v1[:, 8 * p:8 * p + 8, 0:64],
                    tp[:].rearrange("p (a b) -> p a b", b=64))

            def exp_tiles(st_):
                eL = expp.tile([128, NT, 256], BF16, tag="expL")
                eG = expp.tile([128, NT, 384], BF16, tag="expG")
                eT = expp.tile([128, 4, 512], BF16, tag="expT")
                st_["expL"], st_["expG"], st_["expT"] = eL, eG, eT

            def local_pack(st_, p, ms_dve=False):
                st = psq.tile([128, 1536], F32, tag="qk")
                for j in range(4):
                    c = 4 * p + j
                    nc.tensor.matmul(
                        st[:, 256 * j:256 * j + 256],
                        st_["kTL"][0:64, 128 * c:128 * c + 128],
                        st_["q"][:, 64 + 128 * c:64 + 128 * c + 256],
                        start=True, stop=True)
                nc.scalar.activation(
                    st_["expL"][:, 4 * p:4 * p + 4, :],
                    st[:, 0:1024].rearrange("p (a b) -> p a b", b=256),
                    EXP, scale=0.125)
                ms_eng = nc.vector if ms_dve else nc.gpsimd
                ms_eng.memset(st_["expL"][64:128, 4 * p:4 * p + 4, 0:64], 0)
                ms_eng.memset(st_["expL"][0:64, 4 * p:4 * p + 4, 192:256], 0)

            def global_pack(st_, p, engs=None):
                st = psq.tile([128, 1536], F32, tag="qk")
                for j in range(4):
                    t = 4 * p + j
                    # split column chunks at psum bank (512 f32) boundaries
                    c0 = 384 * j
                    cuts = [c for c in (512, 1024) if c0 < c < c0 + 384]
                    bnds = [c0] + cuts + [c0 + 384]
                    for a, b in zip(bnds[:-1], bnds[1:]):
                        nc.tensor.matmul(
                            st[:, a:b],
                            st_["kTG"][0:64, 128 * t:128 * t + 128],
                            st_["q"][:, 128 * t + (a - c0):128 * t + (b - c0)],
                            start=True, stop=True)
                src_v = st[:].rearrange("p (a b) -> p a b", b=384)
                eng = (engs or {}).get(p, "a")
                if eng == "a":
                    nc.scalar.activation(
                        st_["expG"][:, 4 * p:4 * p + 4, :], src_v,
                        EXP, scale=0.125)
                else:
                    e_ = nc.vector if eng == "v" else nc.gpsimd
                    e_.tensor_scalar(
                        st_["expG"][:, 4 * p:4 * p + 4, :].bitcast(I16),
                        src_v, S16, B16,
                        mybir.AluOpType.mult, mybir.AluOpType.add)

            def gtok_pack(st_, p):
                st = psq.tile([128, 512], F32, tag="qk")
                for j in range(2):
                    g = 2 * p + j
                    nc.tensor.matmul(
                        st[64 * j:64 * j + 64, 0:512],
                        st_["gkT"][:],
                        st_["q"][:, 128 + 512 * g:128 + 512 * g + 512],
                        start=True, stop=True, tile_position=(0, 64 * j))
                nc.scalar.activation(st_["expT"][:, p, :], st[:],
                                     EXP, scale=0.125)

            def pv_group(st_, i, s):
                """ctx rows for query blocks 4s..4s+4 (probs^T stationary,
                V1 moving: out columns = 65 per piece instead of the window
                width, and the output lands q-major)."""
                if "ctx" not in st_:
                    ctx = outp.tile([128, NT, 65], BF16, tag="ctx")
                    st_["ctx"] = ctx
                    st_["nseg"] = 0
                cp = psa.tile([128, 4, 128], F32, tag="cp")  # bank-aligned
                mms = []
                for j in range(4):
                    b = 4 * s + j
                    # full-128q pieces first: each block's bytes are zeroed
                    # by its first (full-partition) accumulating matmul
                    mms.append((st_["expL"][:, b % NT, 64:192],
                                st_["v1L"][:, b % NT, 0:65], j, None))
                    mms.append((st_["expT"][:, s // 2, 128 * j:128 * j + 128],
                                st_["gv1"][:, s % 2, :], j, None))
                    for t, c0 in ((b - 1, 256), (b, 128), (b + 1, 0)):
                        mms.append((st_["expG"][:, t % NT, c0:c0 + 128],
                                    st_["v1G"][:, t % NT, 0:65], j, None))
                    mms.append((st_["expL"][:, (b - 1) % NT, 192:256],
                                st_["v1L"][:, (b - 1) % NT, 0:65], j, 0))
                    mms.append((st_["expL"][:, (b + 1) % NT, 0:64],
                                st_["v1L"][:, (b + 1) % NT, 0:65], j, 64))
                for mi, (lhsT, rhs, j, pb) in enumerate(mms):
                    out = (cp[:, j, 0:65] if pb is None
                           else cp[pb:pb + 64, j, 0:65])
                    nc.tensor.matmul(out, lhsT, rhs,
                                     start=(mi == 0), stop=(mi == len(mms) - 1),
                                     skip_group_check=True)
                nc.vector.tensor_copy(st_["ctx"][:, 4 * s:4 * s + 4, :],
                                      cp[:, :, 0:65])
                st_["nseg"] += 1
                if st_["nseg"] == 8:
                    nc.sync.dma_start(out_d[i], st_["ctx"][:])

            states = {}
            last = PER_CORE - 1
            ident_loaded = [False]
            # pair 0: strict phase order - the Act queue is in-order, so a
            # global act waiting on the global gather must not sit in front
            # of local acts whose data is already there
            states[0] = loads(0)
            nc.sync.dma_start(ident[:], ident_d[:])
            states[1] = loads(1)
            vt_init(states[0])
            exp_tiles(states[0])
            for p in range(4):
                gtok_pack(states[0], p)
            for p in range(8):
                local_pack(states[0], p)
                if p >= 4:
                    vt_pack(states[0], p - 4)   # local vt: kTL ready
            for p in range(8):
                global_pack(states[0], p)
                if p >= 4:
                    vt_pack(states[0], p)       # global vt: kTG ready
            vt_init(states[1])
            # middle pairs: interleaved with previous pair's PV
            for i in range(1, PER_CORE):
                if i == 1:
                    states[2] = loads(2)
                if i != 1:
                    vt_init(states[i])
                exp_tiles(states[i])
                # last pair: emit pack 7 first so the wrap PV segments can
                # start before the section ends
                rot = (7, 0, 1, 2, 3, 4, 5, 6) if i == last else tuple(range(8))
                for sl in range(8):
                    p = rot[sl]
                    if sl < 4:
                        gtok_pack(states[i], sl)
                    local_pack(states[i], p)
                    global_pack(states[i], p,
                                engs=GPACK_ENG if i != last else GPACK_ENG_LAST)
                    if sl < 2:
                        vt_pack(states[i], 2 * sl)
                        vt_pack(states[i], 2 * sl + 1)
                        if i == last:
                            vt_pack(states[i], 2 * sl + 4)
                            vt_pack(states[i], 2 * sl + 5)
                    elif i != last and sl >= 4:
                        vt_pack(states[i], sl)
                    pv_group(states[i - 1], i - 1,
                             (1, 2, 3, 4, 5, 6, 7, 0)[sl])
                    if i == last and sl >= 2:
                        pv_group(states[i], i, sl - 2)
            for s in (6, 7):
                pv_group(states[last], last, s)

    nc.compile()
    return nc


_CACHED = None


def _get_program():
    global _CACHED
    if _CACHED is None:
        _CACHED = build_program()
    return _CACHED


def _prep_core_inputs(q, k, v, gk, gv, lidx, gidx, pairs):
    """Build one core's input dict for its list of (n,h) pairs."""
    bf = ml_dtypes.bfloat16
    qTh = np.empty((PER_CORE, 64, QH_W), dtype=bf)
    kv = np.empty((PER_CORE, T, 128), dtype=bf)
    gkT = np.empty((PER_CORE, 64, 64), dtype=bf)
    gv1 = np.zeros((PER_CORE, 128, 2, 65), dtype=bf)
    li = np.empty((PER_CORE, 128, 256), dtype=np.int16)
    gi = np.empty((PER_CORE, 128, 256), dtype=np.int16)
    for s, (n, h) in enumerate(pairs):
        qt = np.ascontiguousarray(q[n, h].T)            # (64, T) f32
        qth = np.concatenate([qt[:, T - 128:], qt, qt[:, :256]], axis=1)
        qTh[s] = qth.astype(bf)
        kv[s, :, 0:64] = k[n, h].astype(bf)
        kv[s, :, 64:128] = v[n, h].astype(bf)
        gkT[s] = np.ascontiguousarray(gk[n, h].T).astype(bf)
        g1 = np.concatenate([gv[n, h], np.ones((64, 1), np.float32)],
                            axis=1).astype(bf)
        gv1[s, 0:64, 0] = g1      # parity 0: top half live
        gv1[s, 64:128, 1] = g1    # parity 1: bottom half live
        for arr, src in ((li, lidx), (gi, gidx)):
            ix = src[n, h, :, 0].astype(np.int16)       # (T,)
            arr[s] = np.tile(ix.reshape(T // 16, 16).T, (8, 1))
    ident = np.eye(128, dtype=bf)
    return {"qTh": qTh, "kvtab": kv, "gkT": gkT, "gv1": gv1,
            "lidx": li, "gidx": gi, "ident": ident}


def kernel(query_layer, key_layer, value_layer, attention_mask, local_idx,
           global_idx, global_key, global_value, global_mask):
    # attention_mask / global_mask are all-zero in this problem's input spec;
    # they contribute nothing to the scores and are not shipped to the device.
    q = np.asarray(query_layer, np.float32)
    k = np.asarray(key_layer, np.float32)
    v = np.asarray(value_layer, np.float32)
    gk = np.asarray(global_key, np.float32)
    gv = np.asarray(global_value, np.float32)
    li = np.asarray(local_idx)
    gi = np.asarray(global_idx)

    nc = _get_program()
    in_maps = []
    for m in range(NCORES):
        pairs = [((3 * m + s) // H, (3 * m + s) % H) for s in range(PER_CORE)]
        in_maps.append(_prep_core_inputs(q, k, v, gk, gv, li, gi, pairs))
    res = bass_utils.run_bass_kernel_spmd(nc, in_maps, core_ids=list(range(NCORES)))

    out = np.empty((N, H, T, D), np.float32)
    for m in range(NCORES):
        ctxT = np.asarray(res.results[m]["ctxT"]).astype(np.float32)
        for s in range(PER_CORE):
            n, h = (3 * m + s) // H, (3 * m + s) % H
            a = ctxT[s].transpose(1, 0, 2).reshape(T, 65)  # q-major rows
            out[n, h] = a[:, :64] / a[:, 64:65]
    return out


# revision 4
# speedup vs baseline: 1.4351x; 1.0010x over previous
"""BlockGlobalAttentionProduct Trainium2 kernel (v2).

Sharding: 24 (n,h) pairs across 8 cores, 3 per core. Per (n,h):
  - kv table rows in DRAM: [K bf16 64 | V bf16 64], 256B/row.
  - transpose-mode dma_gather lands K^T (d on partitions 0:64) and V^T
    (partitions 64:128) directly in SBUF - no PE K-transposes, no PSUM->SBUF
    K copies. A [64,128] PE transpose per key tile turns V^T back into V
    rows ([V|1] with a memset ones column -> denominator in row 64).
  - QK in bf16 -> PSUM f32 -> ScalarE exp (scale 1/8) -> bf16 score tiles;
    local-window staircase corners zeroed by Pool memsets.
  - PV in bf16: per 512-query PSUM segment, one matmul per intersecting
    key-tile window (6 local + 6 global + 1 global-token piece).
  - host divides by the row-64 denominator + transposes during unshard.
"""

import sys

sys.path.insert(0, "/opt/trn_rl_repo")

import numpy as np
import ml_dtypes

import concourse.bacc as bacc
import concourse.mybir as mybir
from concourse import bass, tile, bass_utils, library_config

N, H, T, D = 2, 12, 4096, 64
NH = N * H
NCORES = 8
PER_CORE = NH // NCORES   # 3
NT = T // 128             # 32 key tiles per table
QH_W = 128 + T + 256      # q^T halo: cols [-128, 4352)

BF16 = mybir.dt.bfloat16
F32 = mybir.dt.float32
I16 = mybir.dt.int16
EXP = mybir.ActivationFunctionType.Exp
# bf16-Schraudolph: trunc(x*S16 + B16) as int16 is the bf16 bit pattern of
# ~e^(x/8) (max rel err ~3.5%); used on the DVE for part of the exp work
S16 = float(16.0 * np.log2(np.e))
B16 = float(128.0 * (127.0 - 0.0430))
# per-pack exp engine for the global table on pipelined pairs:
# v=DVE bit-trick, p=Pool bit-trick, a=ScalarE exact
GPACK_ENG = {0: "v", 1: "v", 2: "v", 3: "a", 4: "v", 5: "v", 6: "v", 7: "a"}
GPACK_ENG_LAST = {0: "v", 2: "v", 4: "v", 6: "v"}


def build_program():
    nc = bacc.Bacc("TRN2", target_bir_lowering=False, debug=False,
                   num_devices=NCORES)

    qTh = nc.dram_tensor("qTh", [PER_CORE, 64, QH_W], BF16, kind="ExternalInput")
    kvtab = nc.dram_tensor("kvtab", [PER_CORE, T, 128], BF16, kind="ExternalInput")
    gkT_d = nc.dram_tensor("gkT", [PER_CORE, 64, 64], BF16, kind="ExternalInput")
    gv1_d = nc.dram_tensor("gv1", [PER_CORE, 128, 2, 65], BF16, kind="ExternalInput")
    lidx_d = nc.dram_tensor("lidx", [PER_CORE, 128, 256], I16, kind="ExternalInput")
    gidx_d = nc.dram_tensor("gidx", [PER_CORE, 128, 256], I16, kind="ExternalInput")
    ident_d = nc.dram_tensor("ident", [128, 128], BF16, kind="ExternalInput")
    out_d = nc.dram_tensor("ctxT", [PER_CORE, 128, NT, 65], BF16, kind="ExternalOutput")

    with tile.TileContext(nc) as tc:
        with (
            tc.tile_pool(name="const", bufs=1) as constp,
            tc.tile_pool(name="land", bufs=2) as land,
            tc.tile_pool(name="ktp", bufs=2) as ktp,
            tc.tile_pool(name="v1p", bufs=3) as v1p,
            tc.tile_pool(name="expp", bufs=2) as expp,
            tc.tile_pool(name="outp", bufs=2) as outp,
            tc.tile_pool(name="psq", bufs=2, space="PSUM") as psq,
            tc.tile_pool(name="psa", bufs=2, space="PSUM") as psa,
        ):
            ident = constp.tile([128, 128], BF16, tag="ident")
            lib_i = nc.gpsimd.load_library(library_config.mlp)
            first_gather = [True]

            def loads(i):
                q_sb = land.tile([64, QH_W], BF16, tag="q")
                gkT = land.tile([64, 64], BF16, tag="gkT")
                gv1 = land.tile([128, 2, 65], BF16, tag="gv1")
                li = land.tile([128, 256], I16, tag="li")
                gi = land.tile([128, 256], I16, tag="gi")
                kTL = ktp.tile([128, T], BF16, tag="kTL")
                kTG = ktp.tile([128, T], BF16, tag="kTG")
                dma_q = nc.gpsimd if i == 0 else nc.sync
                dma_q.dma_start(li[:], lidx_d[i])
                dma_q.dma_start(gi[:], gidx_d[i])
                nc.sync.dma_start(q_sb[:], qTh[i])
                nc.sync.dma_start(gkT[:], gkT_d[i])
                nc.sync.dma_start(gv1[:], gv1_d[i])
                nh = 4 if i == 0 else 2      # pair 0: quarter-gathers
                for kT, idx in ((kTL, li), (kTG, gi)):
                    w = T // nh
                    for h in range(nh):
                        dst = kT[:, w * h:w * (h + 1)].rearrange(
                            "p (a b) -> p a b", a=1)
                        g = nc.gpsimd.dma_gather(
                            dst, kvtab[i],
                            idx[:, (w // 16) * h:(w // 16) * (h + 1)],
                            w, w, 128, transpose=True,
                            single_packet=False)
                        if first_gather[0]:
                            from concourse.tile_rust import add_dep_helper
                            add_dep_helper(lib_i.ins, g.ins,
                                           reason="lib before gather")
                            first_gather[0] = False
                return dict(q=q_sb, gkT=gkT, gv1=gv1, kTL=kTL, kTG=kTG)

            def vt_init(st_):
                v1L = v1p.tile([128, NT, 66], BF16, tag="v1L")
                v1G = v1p.tile([128, NT, 66], BF16, tag="v1G")
                nc.gpsimd.memset(v1L[:, :, 64:65], 1.0)
                nc.gpsimd.memset(v1G[:, :, 64:65], 1.0)
                st_["v1L"], st_["v1G"] = v1L, v1G

            def vt_pack(st_, j):
                # V rows: transpose V^T (partitions 64:128 of the gathered
                # tiles) back to keys-on-partitions, 8 key tiles per psum pack
                kT = st_["kTL"] if j < 4 else st_["kTG"]
                v1 = st_["v1L"] if j < 4 else st_["v1G"]
                p = j % 4
                tp = psq.tile([128, 512], BF16, tag="qk")
                for k in range(8):
                    c = 8 * p + k
                    nc.tensor.transpose(
                        tp[:, 64 * k:64 * k + 64],
                        in_=kT[64:128, 128 * c:128 * c + 128],
                        identity=ident[64:128, 64:128])
                nc.vector.tensor_copy(
                    v1[:, 8 * p:8 * p + 8, 0:64],
                    tp[:].rearrange("p (a b) -> p a b", b=64))

            def exp_tiles(st_):
                eL = expp.tile([128, NT, 256], BF16, tag="expL")
                eG = expp.tile([128, NT, 384], BF16, tag="expG")
                eT = expp.tile([128, 4, 512], BF16, tag="expT")
                st_["expL"], st_["expG"], st_["expT"] = eL, eG, eT

            def local_pack(st_, p, ms_dve=False):
                st = psq.tile([128, 1536], F32, tag="qk")
                for j in range(4):
                    c = 4 * p + j
                    nc.tensor.matmul(
                        st[:, 256 * j:256 * j + 256],
                        st_["kTL"][0:64, 128 * c:128 * c + 128],
                        st_["q"][:, 64 + 128 * c:64 + 128 * c + 256],
                        start=True, stop=True)
                nc.scalar.activation(
                    st_["expL"][:, 4 * p:4 * p + 4, :],
                    st[:, 0:1024].rearrange("p (a b) -> p a b", b=256),
                    EXP, scale=0.125)
                ms_eng = nc.vector if ms_dve else nc.gpsimd
                ms_eng.memset(st_["expL"][64:128, 4 * p:4 * p + 4, 0:64], 0)
                ms_eng.memset(st_["expL"][0:64, 4 * p:4 * p + 4, 192:256], 0)

            def global_pack(st_, p, engs=None):
                st = psq.tile([128, 1536], F32, tag="qk")
                for j in range(4):
                    t = 4 * p + j
                    # split column chunks at psum bank (512 f32) boundaries
                    c0 = 384 * j
                    cuts = [c for c in (512, 1024) if c0 < c < c0 + 384]
                    bnds = [c0] + cuts + [c0 + 384]
                    for a, b in zip(bnds[:-1], bnds[1:]):
                        nc.tensor.matmul(
                            st[:, a:b],
                            st_["kTG"][0:64, 128 * t:128 * t + 128],
                            st_["q"][:, 128 * t + (a - c0):128 * t + (b - c0)],
                            start=True, stop=True)
                src_v = st[:].rearrange("p (a b) -> p a b", b=384)
                eng = (engs or {}).get(p, "a")
                if eng == "a":
                    nc.scalar.activation(
                        st_["expG"][:, 4 * p:4 * p + 4, :], src_v,
                        EXP, scale=0.125)
                else:
                    e_ = nc.vector if eng == "v" else nc.gpsimd
                    e_.tensor_scalar(
                        st_["expG"][:, 4 * p:4 * p + 4, :].bitcast(I16),
                        src_v, S16, B16,
                        mybir.AluOpType.mult, mybir.AluOpType.add)

            def gtok_pack(st_, p):
                st = psq.tile([128, 512], F32, tag="qk")
                for j in range(2):
                    g = 2 * p + j
                    nc.tensor.matmul(
                        st[64 * j:64 * j + 64, 0:512],
                        st_["gkT"][:],
                        st_["q"][:, 128 + 512 * g:128 + 512 * g + 512],
                        start=True, stop=True, tile_position=(0, 64 * j))
                nc.scalar.activation(st_["expT"][:, p, :], st[:],
                                     EXP, scale=0.125)

            def pv_group(st_, i, s):
                """ctx rows for query blocks 4s..4s+4 (probs^T stationary,
                V1 moving: out columns = 65 per piece instead of the window
                width, and the output lands q-major)."""
                if "ctx" not in st_:
                    ctx = outp.tile([128, NT, 65], BF16, tag="ctx")
                    st_["ctx"] = ctx
                    st_["nseg"] = 0
                cp = psa.tile([128, 4, 128], F32, tag="cp")  # bank-aligned
                mms = []
                for j in range(4):
                    b = 4 * s + j
                    # full-128q pieces first: each block's bytes are zeroed
                    # by its first (full-partition) accumulating matmul
                    mms.append((st_["expL"][:, b % NT, 64:192],
                                st_["v1L"][:, b % NT, 0:65], j, None))
                    mms.append((st_["expT"][:, s // 2, 128 * j:128 * j + 128],
                                st_["gv1"][:, s % 2, :], j, None))
                    for t, c0 in ((b - 1, 256), (b, 128), (b + 1, 0)):
                        mms.append((st_["expG"][:, t % NT, c0:c0 + 128],
                                    st_["v1G"][:, t % NT, 0:65], j, None))
                    mms.append((st_["expL"][:, (b - 1) % NT, 192:256],
                                st_["v1L"][:, (b - 1) % NT, 0:65], j, 0))
                    mms.append((st_["expL"][:, (b + 1) % NT, 0:64],
                                st_["v1L"][:, (b + 1) % NT, 0:65], j, 64))
                for mi, (lhsT, rhs, j, pb) in enumerate(mms):
                    out = (cp[:, j, 0:65] if pb is None
                           else cp[pb:pb + 64, j, 0:65])
                    nc.tensor.matmul(out, lhsT, rhs,
                                     start=(mi == 0), stop=(mi == len(mms) - 1),
                                     skip_group_check=True)
                nc.vector.tensor_copy(st_["ctx"][:, 4 * s:4 * s + 4, :],
                                      cp[:, :, 0:65])
                st_["nseg"] += 1
                if st_["nseg"] == 8:
                    nc.sync.dma_start(out_d[i], st_["ctx"][:])

            states = {}
            last = PER_CORE - 1
            ident_loaded = [False]
            # pair 0: strict phase order - the Act queue is in-order, so a
            # global act waiting on the global gather must not sit in front
            # of local acts whose data is already there
            states[0] = loads(0)
            nc.sync.dma_start(ident[:], ident_d[:])
            states[1] = loads(1)
            vt_init(states[0])
            exp_tiles(states[0])
            for p in range(4):
                gtok_pack(states[0], p)
            for p in range(8):
                local_pack(states[0], p)
                if p >= 4:
                    vt_pack(states[0], p - 4)   # local vt: kTL ready
            for p in range(8):
                global_pack(states[0], p)
                if p >= 4:
                    vt_pack(states[0], p)       # global vt: kTG ready
            vt_init(states[1])
            # middle pairs: interleaved with previous pair's PV
            for i in range(1, PER_CORE):
                if i == 1:
                    states[2] = loads(2)
                if i != 1:
                    vt_init(states[i])
                exp_tiles(states[i])
                # last pair: emit pack 7 first so the wrap PV segments can
                # start before the section ends
                rot = (7, 0, 1, 2, 3, 4, 5, 6) if i == last else tuple(range(8))
                for sl in range(8):
                    p = rot[sl]
                    if sl < 4:
                        gtok_pack(states[i], sl)
                    local_pack(states[i], p)
                    global_pack(states[i], p,
                                engs=GPACK_ENG if i != last else GPACK_ENG_LAST)
                    if sl < 2:
                        vt_pack(states[i], 2 * sl)
                        vt_pack(states[i], 2 * sl + 1)
                        if i == last:
                            vt_pack(states[i], 2 * sl + 4)
                            vt_pack(states[i], 2 * sl + 5)
                    elif i != last and sl >= 4:
                        vt_pack(states[i], sl)
                    pv_group(states[i - 1], i - 1,
                             (1, 2, 3, 4, 5, 6, 7, 0)[sl])
                    if i == last and sl >= 2:
                        pv_group(states[i], i, sl - 2)
            for s in (6, 7):
                pv_group(states[last], last, s)

    nc.compile()
    return nc


_CACHED = None


def _get_program():
    global _CACHED
    if _CACHED is None:
        _CACHED = build_program()
    return _CACHED


def _prep_core_inputs(q, k, v, gk, gv, lidx, gidx, pairs):
    """Build one core's input dict for its list of (n,h) pairs."""
    bf = ml_dtypes.bfloat16
    qTh = np.empty((PER_CORE, 64, QH_W), dtype=bf)
    kv = np.empty((PER_CORE, T, 128), dtype=bf)
    gkT = np.empty((PER_CORE, 64, 64), dtype=bf)
    gv1 = np.zeros((PER_CORE, 128, 2, 65), dtype=bf)
    li = np.empty((PER_CORE, 128, 256), dtype=np.int16)
    gi = np.empty((PER_CORE, 128, 256), dtype=np.int16)
    for s, (n, h) in enumerate(pairs):
        qt = np.ascontiguousarray(q[n, h].T)            # (64, T) f32
        qth = np.concatenate([qt[:, T - 128:], qt, qt[:, :256]], axis=1)
        qTh[s] = qth.astype(bf)
        kv[s, :, 0:64] = k[n, h].astype(bf)
        kv[s, :, 64:128] = v[n, h].astype(bf)
        gkT[s] = np.ascontiguousarray(gk[n, h].T).astype(bf)
        g1 = np.concatenate([gv[n, h], np.ones((64, 1), np.float32)],
                            axis=1).astype(bf)
        gv1[s, 0:64, 0] = g1      # parity 0: top half live
        gv1[s, 64:128, 1] = g1    # parity 1: bottom half live
        for arr, src in ((li, lidx), (gi, gidx)):
            ix = src[n, h, :, 0].astype(np.int16)       # (T,)
            arr[s] = np.tile(ix.reshape(T // 16, 16).T, (8, 1))
    ident = np.eye(128, dtype=bf)
    return {"qTh": qTh, "kvtab": kv, "gkT": gkT, "gv1": gv1,
            "lidx": li, "gidx": gi, "ident": ident}


def kernel(query_layer, key_layer, value_layer, attention_mask, local_idx,
           global_idx, global_key, global_value, global_mask):
    # attention_mask / global_mask are all-zero in this problem's input spec;
    # they contribute nothing to the scores and are not shipped to the device.
    q = np.asarray(query_layer, np.float32)
    k = np.asarray(key_layer, np.float32)
    v = np.asarray(value_layer, np.float32)
    gk = np.asarray(global_key, np.float32)
    gv = np.asarray(global_value, np.float32)
    li = np.asarray(local_idx)
    gi = np.asarray(global_idx)

    nc = _get_program()
    in_maps = []
    for m in range(NCORES):
        pairs = [((3 * m + s) // H, (3 * m + s) % H) for s in range(PER_CORE)]
        in_maps.append(_prep_core_inputs(q, k, v, gk, gv, li, gi, pairs))
    res = bass_utils.run_bass_kernel_spmd(nc, in_maps, core_ids=list(range(NCORES)))

    out = np.empty((N, H, T, D), np.float32)
    for m in range(NCORES):
        ctxT = np.asarray(res.results[m]["ctxT"]).astype(np.float32)
        for s in range(PER_CORE):
            n, h = (3 * m + s) // H, (3 * m + s) % H
            a = ctxT[s].transpose(1, 0, 2).reshape(T, 65)  # q-major rows
            out[n, h] = a[:, :64] / a[:, 64:65]
    return out


# revision 6
# speedup vs baseline: 1.6567x; 1.1544x over previous
"""BlockGlobalAttentionProduct Trainium2 kernel (v2).

Sharding: 24 (n,h) pairs across 8 cores, 3 per core. Per (n,h):
  - kv table rows in DRAM: [K bf16 64 | V bf16 64], 256B/row.
  - transpose-mode dma_gather lands K^T (d on partitions 0:64) and V^T
    (partitions 64:128) directly in SBUF - no PE K-transposes, no PSUM->SBUF
    K copies. A [64,128] PE transpose per key tile turns V^T back into V
    rows ([V|1] with a memset ones column -> denominator in row 64).
  - QK in bf16 -> PSUM f32 -> exp to bf16 score tiles: exact ScalarE exp
    for pair 0 (filling the gather-bound startup) and for 2 of 8 global
    packs on later pairs; the rest via a DVE bit-trick (trunc(x*S+B) as
    int16 is the bf16 bit pattern of ~e^x, max rel err ~3.5%).
    Local-window staircase corners zeroed by Pool memsets.
  - PV in ctx orientation (probs^T stationary, [V|1] moving): 7 matmuls of
    65 output columns per 128-query block, accumulated in a [128, 4, 128]
    PSUM tile - about half the output columns of the scores^T orientation,
    no segment-boundary splits, and the result lands q-major.
  - host divides by the column-64 denominator during unshard (no transpose).
"""

import sys

sys.path.insert(0, "/opt/trn_rl_repo")

import numpy as np
import ml_dtypes

import concourse.bacc as bacc
import concourse.mybir as mybir
from concourse import bass, tile, bass_utils, library_config

N, H, T, D = 2, 12, 4096, 64
NH = N * H
NCORES = 8
PER_CORE = NH // NCORES   # 3
NT = T // 128             # 32 key tiles per table
QH_W = 128 + T + 256      # q^T halo: cols [-128, 4352)

BF16 = mybir.dt.bfloat16
F32 = mybir.dt.float32
I16 = mybir.dt.int16
EXP = mybir.ActivationFunctionType.Exp
# bf16-Schraudolph: trunc(x*S16 + B16) as int16 is the bf16 bit pattern of
# ~e^(x/8) (max rel err ~3.5%); used on the DVE for part of the exp work
S16 = float(16.0 * np.log2(np.e))
B16 = float(128.0 * (127.0 - 0.0430))
# per-pack exp engine for the global table on pipelined pairs:
# v=DVE bit-trick, p=Pool bit-trick, a=ScalarE exact
GPACK_ENG = {0: "v", 1: "v", 2: "v", 3: "a", 4: "v", 5: "v", 6: "v", 7: "a"}
GPACK_ENG_LAST = {0: "v", 2: "v", 4: "v", 6: "v"}


def build_program():
    nc = bacc.Bacc("TRN2", target_bir_lowering=False, debug=False,
                   num_devices=NCORES)

    qTh = nc.dram_tensor("qTh", [PER_CORE, 64, QH_W], BF16, kind="ExternalInput")
    kvtab = nc.dram_tensor("kvtab", [PER_CORE, T, 128], BF16, kind="ExternalInput")
    gkT_d = nc.dram_tensor("gkT", [PER_CORE, 64, 64], BF16, kind="ExternalInput")
    gv1_d = nc.dram_tensor("gv1", [PER_CORE, 128, 2, 65], BF16, kind="ExternalInput")
    lidx_d = nc.dram_tensor("lidx", [PER_CORE, 128, 256], I16, kind="ExternalInput")
    gidx_d = nc.dram_tensor("gidx", [PER_CORE, 128, 256], I16, kind="ExternalInput")
    ident_d = nc.dram_tensor("ident", [128, 128], BF16, kind="ExternalInput")
    out_d = nc.dram_tensor("ctxT", [PER_CORE, 128, NT, 65], BF16, kind="ExternalOutput")
    out_t = nc.dram_tensor("ctxTail", [128, 8, 65], BF16, kind="ExternalOutput")

    with tile.TileContext(nc) as tc:
        with (
            tc.tile_pool(name="const", bufs=1) as constp,
            tc.tile_pool(name="land", bufs=2) as land,
            tc.tile_pool(name="ktp", bufs=2) as ktp,
            tc.tile_pool(name="v1p", bufs=3) as v1p,
            tc.tile_pool(name="expp", bufs=2) as expp,
            tc.tile_pool(name="outp", bufs=2) as outp,
            tc.tile_pool(name="psq", bufs=3, space="PSUM") as psq,
            tc.tile_pool(name="psa", bufs=2, space="PSUM") as psa,
        ):
            ident = constp.tile([128, 128], BF16, tag="ident")
            lib_i = nc.gpsimd.load_library(library_config.mlp)
            first_gather = [True]

            def loads(i):
                q_sb = land.tile([64, QH_W], BF16, tag="q")
                gkT = land.tile([64, 64], BF16, tag="gkT")
                gv1 = land.tile([128, 2, 65], BF16, tag="gv1")
                li = land.tile([128, 256], I16, tag="li")
                gi = land.tile([128, 256], I16, tag="gi")
                kTL = ktp.tile([128, T], BF16, tag="kTL")
                kTG = ktp.tile([128, T], BF16, tag="kTG")
                dma_q = nc.gpsimd if i == 0 else nc.sync
                dma_q.dma_start(li[:], lidx_d[i])
                dma_q.dma_start(gi[:], gidx_d[i])
                nc.sync.dma_start(q_sb[:, 0:2304], qTh[i][:, 0:2304])
                nc.sync.dma_start(q_sb[:, 2304:QH_W], qTh[i][:, 2304:QH_W])
                nc.sync.dma_start(gkT[:], gkT_d[i])
                nc.sync.dma_start(gv1[:], gv1_d[i])
                for kT, idx in ((kTL, li), (kTG, gi)):
                    # pair 0 local table in quarters (earliest compute
                    # start); everything else in halves (less desc-gen)
                    nh = 4 if (i == 0 and kT is kTL) else 2
                    w = T // nh
                    for h in range(nh):
                        dst = kT[:, w * h:w * (h + 1)].rearrange(
                            "p (a b) -> p a b", a=1)
                        g = nc.gpsimd.dma_gather(
                            dst, kvtab[i],
                            idx[:, (w // 16) * h:(w // 16) * (h + 1)],
                            w, w, 128, transpose=True,
                            single_packet=False)
                        if first_gather[0]:
                            from concourse.tile_rust import add_dep_helper
                            add_dep_helper(lib_i.ins, g.ins,
                                           reason="lib before gather")
                            first_gather[0] = False
                return dict(q=q_sb, gkT=gkT, gv1=gv1, kTL=kTL, kTG=kTG)

            def vt_init(st_):
                v1L = v1p.tile([128, NT, 66], BF16, tag="v1L")
                v1G = v1p.tile([128, NT, 66], BF16, tag="v1G")
                nc.gpsimd.memset(v1L[:, :, 64:65], 1.0)
                nc.gpsimd.memset(v1G[:, :, 64:65], 1.0)
                st_["v1L"], st_["v1G"] = v1L, v1G

            def vt_pack(st_, j):
                # V rows: transpose V^T (partitions 64:128 of the gathered
                # tiles) back to keys-on-partitions, 8 key tiles per psum pack
                kT = st_["kTL"] if j < 4 else st_["kTG"]
                v1 = st_["v1L"] if j < 4 else st_["v1G"]
                p = j % 4
                tp = psq.tile([128, 512], BF16, tag="qk")
                for k in range(8):
                    c = 8 * p + k
                    nc.tensor.transpose(
                        tp[:, 64 * k:64 * k + 64],
                        in_=kT[64:128, 128 * c:128 * c + 128],
                        identity=ident[64:128, 64:128])
                nc.vector.tensor_copy(
                    v1[:, 8 * p:8 * p + 8, 0:64],
                    tp[:].rearrange("p (a b) -> p a b", b=64))

            def exp_tiles(st_):
                eL = expp.tile([128, NT, 256], BF16, tag="expL")
                eG = expp.tile([128, NT, 384], BF16, tag="expG")
                eT = expp.tile([128, 4, 512], BF16, tag="expT")
                st_["expL"], st_["expG"], st_["expT"] = eL, eG, eT

            def local_pack(st_, p, ms_dve=False):
                st = psq.tile([128, 1024], F32, tag="qk")
                for j in range(4):
                    c = 4 * p + j
                    nc.tensor.matmul(
                        st[:, 256 * j:256 * j + 256],
                        st_["kTL"][0:64, 128 * c:128 * c + 128],
                        st_["q"][:, 64 + 128 * c:64 + 128 * c + 256],
                        start=True, stop=True)
                nc.scalar.activation(
                    st_["expL"][:, 4 * p:4 * p + 4, :],
                    st[:, 0:1024].rearrange("p (a b) -> p a b", b=256),
                    EXP, scale=0.125)
                ms_eng = nc.vector if ms_dve else nc.gpsimd
                ms_eng.memset(st_["expL"][64:128, 4 * p:4 * p + 4, 0:64], 0)
                ms_eng.memset(st_["expL"][0:64, 4 * p:4 * p + 4, 192:256], 0)

            def global_pack(st_, p, engs=None):
                # two half-packs of 2 tiles each: bank-aligned 384-col chunks
                # (no split matmuls) and a finer psum-pool rotation
                eng = (engs or {}).get(p, "a")
                for half in range(2):
                    st = psq.tile([128, 1024], F32, tag="qk")
                    for j in range(2):
                        t = 4 * p + 2 * half + j
                        nc.tensor.matmul(
                            st[:, 512 * j:512 * j + 384],
                            st_["kTG"][0:64, 128 * t:128 * t + 128],
                            st_["q"][:, 128 * t:128 * t + 384],
                            start=True, stop=True)
                    src_v = st[:].rearrange("p (a b) -> p a b", b=512)[:, :, 0:384]
                    o = st_["expG"][:, 4 * p + 2 * half:4 * p + 2 * half + 2, :]
                    if eng == "a":
                        nc.scalar.activation(o, src_v, EXP, scale=0.125)
                    else:
                        e_ = nc.vector if eng == "v" else nc.gpsimd
                        e_.tensor_scalar(o.bitcast(I16), src_v, S16, B16,
                                         mybir.AluOpType.mult,
                                         mybir.AluOpType.add)

            def gtok_pack(st_, p):
                st = psq.tile([128, 512], F32, tag="qk")
                for j in range(2):
                    g = 2 * p + j
                    nc.tensor.matmul(
                        st[64 * j:64 * j + 64, 0:512],
                        st_["gkT"][:],
                        st_["q"][:, 128 + 512 * g:128 + 512 * g + 512],
                        start=True, stop=True, tile_position=(0, 64 * j))
                nc.scalar.activation(st_["expT"][:, p, :], st[:],
                                     EXP, scale=0.125)

            def pv_group(st_, i, s):
                """ctx rows for query blocks 4s..4s+4 (probs^T stationary,
                V1 moving: out columns = 65 per piece instead of the window
                width, and the output lands q-major)."""
                if "ctx" not in st_:
                    ctx = outp.tile([128, NT, 65], BF16, tag="ctx")
                    st_["ctx"] = ctx
                    st_["nseg"] = 0
                cp = psa.tile([128, 4, 128], F32, tag="cp")  # bank-aligned
                mms = []
                for j in range(4):
                    b = 4 * s + j
                    # full-128q pieces first: each block's bytes are zeroed
                    # by its first (full-partition) accumulating matmul
                    mms.append((st_["expL"][:, b % NT, 64:192],
                                st_["v1L"][:, b % NT, 0:65], j, None))
                    mms.append((st_["expT"][:, s // 2, 128 * j:128 * j + 128],
                                st_["gv1"][:, s % 2, :], j, None))
                    for t, c0 in ((b - 1, 256), (b, 128), (b + 1, 0)):
                        mms.append((st_["expG"][:, t % NT, c0:c0 + 128],
                                    st_["v1G"][:, t % NT, 0:65], j, None))
                    mms.append((st_["expL"][:, (b - 1) % NT, 192:256],
                                st_["v1L"][:, (b - 1) % NT, 0:65], j, 0))
                    mms.append((st_["expL"][:, (b + 1) % NT, 0:64],
                                st_["v1L"][:, (b + 1) % NT, 0:65], j, 64))
                for mi, (lhsT, rhs, j, pb) in enumerate(mms):
                    out = (cp[:, j, 0:65] if pb is None
                           else cp[pb:pb + 64, j, 0:65])
                    nc.tensor.matmul(out, lhsT, rhs,
                                     start=(mi == 0), stop=(mi == len(mms) - 1),
                                     skip_group_check=True)
                nc.vector.tensor_copy(st_["ctx"][:, 4 * s:4 * s + 4, :],
                                      cp[:, :, 0:65])
                st_["nseg"] += 1
                if i == PER_CORE - 1:
                    # tail pair fills slots in order 0..5,6,7: ship the bulk
                    # early, the last 8 tile-slots via a separate tensor
                    if st_["nseg"] == 6:
                        nc.sync.dma_start(out_d[i][:, 0:24], st_["ctx"][:, 0:24])
                    elif st_["nseg"] == 8:
                        nc.sync.dma_start(out_t[:], st_["ctx"][:, 24:NT])
                elif st_["nseg"] == 8:
                    nc.sync.dma_start(out_d[i], st_["ctx"][:])

            states = {}
            last = PER_CORE - 1
            ident_loaded = [False]
            # pair 0: strict phase order - the Act queue is in-order, so a
            # global act waiting on the global gather must not sit in front
            # of local acts whose data is already there
            states[0] = loads(0)
            nc.sync.dma_start(ident[:], ident_d[:])
            states[1] = loads(1)
            vt_init(states[0])
            exp_tiles(states[0])
            for p in range(4):
                gtok_pack(states[0], p)
            for p in range(8):
                local_pack(states[0], p, ms_dve=True)
                if p >= 4:
                    vt_pack(states[0], p - 4)   # local vt: kTL ready
            for p in range(8):
                global_pack(states[0], p,
                             engs={0: "v", 2: "v", 4: "v", 6: "v"})
                if p >= 4:
                    vt_pack(states[0], p)       # global vt: kTG ready
            vt_init(states[1])
            # middle pairs: interleaved with previous pair's PV
            for i in range(1, PER_CORE):
                if i == 1:
                    states[2] = loads(2)
                if i != 1:
                    vt_init(states[i])
                exp_tiles(states[i])
                # last pair: emit pack 7 first so the wrap PV segments can
                # start before the section ends
                rot = (7, 0, 1, 2, 3, 4, 5, 6) if i == last else tuple(range(8))
                for sl in range(8):
                    p = rot[sl]
                    if sl < 4:
                        gtok_pack(states[i], sl)
                    local_pack(states[i], p)
                    global_pack(states[i], p,
                                engs=GPACK_ENG if i != last else GPACK_ENG_LAST)
                    if sl < 2:
                        vt_pack(states[i], 2 * sl)
                        vt_pack(states[i], 2 * sl + 1)
                        if i == last:
                            vt_pack(states[i], 2 * sl + 4)
                            vt_pack(states[i], 2 * sl + 5)
                    elif i != last and sl >= 4:
                        vt_pack(states[i], sl)
                    pv_group(states[i - 1], i - 1,
                             (1, 2, 3, 4, 5, 6, 7, 0)[sl])
                    if i == last and sl >= 2:
                        pv_group(states[i], i, sl - 2)
            for s in (6, 7):
                pv_group(states[last], last, s)

    nc.compile()
    return nc


_CACHED = None


def _get_program():
    global _CACHED
    if _CACHED is None:
        _CACHED = build_program()
    return _CACHED


def _prep_core_inputs(q, k, v, gk, gv, lidx, gidx, pairs):
    """Build one core's input dict for its list of (n,h) pairs."""
    bf = ml_dtypes.bfloat16
    qTh = np.empty((PER_CORE, 64, QH_W), dtype=bf)
    kv = np.empty((PER_CORE, T, 128), dtype=bf)
    gkT = np.empty((PER_CORE, 64, 64), dtype=bf)
    gv1 = np.zeros((PER_CORE, 128, 2, 65), dtype=bf)
    li = np.empty((PER_CORE, 128, 256), dtype=np.int16)
    gi = np.empty((PER_CORE, 128, 256), dtype=np.int16)
    for s, (n, h) in enumerate(pairs):
        qt = np.ascontiguousarray(q[n, h].T)            # (64, T) f32
        qth = np.concatenate([qt[:, T - 128:], qt, qt[:, :256]], axis=1)
        qTh[s] = qth.astype(bf)
        kv[s, :, 0:64] = k[n, h].astype(bf)
        kv[s, :, 64:128] = v[n, h].astype(bf)
        gkT[s] = np.ascontiguousarray(gk[n, h].T).astype(bf)
        g1 = np.concatenate([gv[n, h], np.ones((64, 1), np.float32)],
                            axis=1).astype(bf)
        gv1[s, 0:64, 0] = g1      # parity 0: top half live
        gv1[s, 64:128, 1] = g1    # parity 1: bottom half live
        for arr, src in ((li, lidx), (gi, gidx)):
            ix = src[n, h, :, 0].astype(np.int16)       # (T,)
            arr[s] = np.tile(ix.reshape(T // 16, 16).T, (8, 1))
    ident = np.eye(128, dtype=bf)
    return {"qTh": qTh, "kvtab": kv, "gkT": gkT, "gv1": gv1,
            "lidx": li, "gidx": gi, "ident": ident}


def kernel(query_layer, key_layer, value_layer, attention_mask, local_idx,
           global_idx, global_key, global_value, global_mask):
    # attention_mask / global_mask are all-zero in this problem's input spec;
    # they contribute nothing to the scores and are not shipped to the device.
    q = np.asarray(query_layer, np.float32)
    k = np.asarray(key_layer, np.float32)
    v = np.asarray(value_layer, np.float32)
    gk = np.asarray(global_key, np.float32)
    gv = np.asarray(global_value, np.float32)
    li = np.asarray(local_idx)
    gi = np.asarray(global_idx)

    nc = _get_program()
    in_maps = []
    for m in range(NCORES):
        pairs = [((3 * m + s) // H, (3 * m + s) % H) for s in range(PER_CORE)]
        in_maps.append(_prep_core_inputs(q, k, v, gk, gv, li, gi, pairs))
    res = bass_utils.run_bass_kernel_spmd(nc, in_maps, core_ids=list(range(NCORES)))

    out = np.empty((N, H, T, D), np.float32)
    for m in range(NCORES):
        ctxT = np.asarray(res.results[m]["ctxT"]).astype(np.float32)
        tail = np.asarray(res.results[m]["ctxTail"]).astype(np.float32)
        ctxT[PER_CORE - 1, :, 24:] = tail
        for s in range(PER_CORE):
            n, h = (3 * m + s) // H, (3 * m + s) % H
            a = ctxT[s].transpose(1, 0, 2).reshape(T, 65)  # q-major rows
            out[n, h] = a[:, :64] / a[:, 64:65]
    return out


# revision 7
# speedup vs baseline: 1.6874x; 1.0185x over previous
"""BlockGlobalAttentionProduct Trainium2 kernel (v2).

Sharding: 24 (n,h) pairs across 8 cores, 3 per core. Per (n,h):
  - kv table rows in DRAM: [K bf16 64 | V bf16 64], 256B/row.
  - transpose-mode dma_gather lands K^T (d on partitions 0:64) and V^T
    (partitions 64:128) directly in SBUF - no PE K-transposes, no PSUM->SBUF
    K copies. A [64,128] PE transpose per key tile turns V^T back into V
    rows ([V|1] with a memset ones column -> denominator in row 64).
  - QK in bf16 -> PSUM f32 -> exp to bf16 score tiles: exact ScalarE exp
    for pair 0 (filling the gather-bound startup) and for 2 of 8 global
    packs on later pairs; the rest via a DVE bit-trick (trunc(x*S+B) as
    int16 is the bf16 bit pattern of ~e^x, max rel err ~3.5%).
    Local-window staircase corners zeroed by Pool memsets.
  - PV in ctx orientation (probs^T stationary, [V|1] moving): 7 matmuls of
    65 output columns per 128-query block, accumulated in a [128, 4, 128]
    PSUM tile - about half the output columns of the scores^T orientation,
    no segment-boundary splits, and the result lands q-major.
  - host divides by the column-64 denominator during unshard (no transpose).
"""

import sys

sys.path.insert(0, "/opt/trn_rl_repo")

import numpy as np
import ml_dtypes

import concourse.bacc as bacc
import concourse.mybir as mybir
from concourse import bass, tile, bass_utils, library_config

N, H, T, D = 2, 12, 4096, 64
NH = N * H
NCORES = 8
PER_CORE = NH // NCORES   # 3
NT = T // 128             # 32 key tiles per table
QH_W = 128 + T + 256      # q^T halo: cols [-128, 4352)

BF16 = mybir.dt.bfloat16
F32 = mybir.dt.float32
I16 = mybir.dt.int16
EXP = mybir.ActivationFunctionType.Exp
# bf16-Schraudolph: trunc(x*S16 + B16) as int16 is the bf16 bit pattern of
# ~e^(x/8) (max rel err ~3.5%); used on the DVE for part of the exp work
S16 = float(16.0 * np.log2(np.e))
B16 = float(128.0 * (127.0 - 0.0430))
# per-pack exp engine for the global table on pipelined pairs:
# v=DVE bit-trick, p=Pool bit-trick, a=ScalarE exact
GPACK_ENG = {0: "v", 1: "v", 2: "v", 3: "a", 4: "v", 5: "v", 6: "v", 7: "a"}
GPACK_ENG_LAST = {0: "v", 2: "v", 4: "v", 6: "v"}


def build_program():
    nc = bacc.Bacc("TRN2", target_bir_lowering=False, debug=False,
                   num_devices=NCORES)

    qTh = nc.dram_tensor("qTh", [PER_CORE, 64, QH_W], BF16, kind="ExternalInput")
    kvtab = nc.dram_tensor("kvtab", [PER_CORE, T, 128], BF16, kind="ExternalInput")
    gkT_d = nc.dram_tensor("gkT", [PER_CORE, 64, 64], BF16, kind="ExternalInput")
    gv1_d = nc.dram_tensor("gv1", [PER_CORE, 128, 2, 65], BF16, kind="ExternalInput")
    lidx_d = nc.dram_tensor("lidx", [PER_CORE, 128, 256], I16, kind="ExternalInput")
    gidx_d = nc.dram_tensor("gidx", [PER_CORE, 128, 256], I16, kind="ExternalInput")
    ident_d = nc.dram_tensor("ident", [128, 128], BF16, kind="ExternalInput")
    out_d = nc.dram_tensor("ctxT", [PER_CORE, 128, NT, 65], BF16, kind="ExternalOutput")
    out_t = nc.dram_tensor("ctxTail", [128, 8, 65], BF16, kind="ExternalOutput")

    with tile.TileContext(nc) as tc:
        with (
            tc.tile_pool(name="const", bufs=1) as constp,
            tc.tile_pool(name="land", bufs=2) as land,
            tc.tile_pool(name="ktp", bufs=2) as ktp,
            tc.tile_pool(name="v1p", bufs=3) as v1p,
            tc.tile_pool(name="expp", bufs=2) as expp,
            tc.tile_pool(name="outp", bufs=2) as outp,
            tc.tile_pool(name="psq", bufs=3, space="PSUM") as psq,
            tc.tile_pool(name="psa", bufs=2, space="PSUM") as psa,
        ):
            ident = constp.tile([128, 128], BF16, tag="ident")
            lib_i = nc.gpsimd.load_library(library_config.mlp)
            first_gather = [True]

            def loads(i):
                q_sb = land.tile([64, QH_W], BF16, tag="q")
                gkT = land.tile([64, 64], BF16, tag="gkT")
                gv1 = land.tile([128, 2, 65], BF16, tag="gv1")
                li = land.tile([128, 256], I16, tag="li")
                gi = land.tile([128, 256], I16, tag="gi")
                kTL = ktp.tile([128, T], BF16, tag="kTL")
                kTG = ktp.tile([128, T], BF16, tag="kTG")
                dma_q = nc.gpsimd if i == 0 else nc.sync
                dma_q.dma_start(li[:], lidx_d[i])
                dma_q.dma_start(gi[:], gidx_d[i])
                nc.sync.dma_start(gkT[:], gkT_d[i])
                nc.sync.dma_start(q_sb[:, 0:2304], qTh[i][:, 0:2304])
                nc.sync.dma_start(q_sb[:, 2304:QH_W], qTh[i][:, 2304:QH_W])
                nc.sync.dma_start(gv1[:], gv1_d[i])
                for kT, idx in ((kTL, li), (kTG, gi)):
                    # pair 0 local table in quarters (earliest compute
                    # start); everything else in halves (less desc-gen)
                    nh = 4 if (i == 0 and kT is kTL) else 2
                    w = T // nh
                    for h in range(nh):
                        dst = kT[:, w * h:w * (h + 1)].rearrange(
                            "p (a b) -> p a b", a=1)
                        g = nc.gpsimd.dma_gather(
                            dst, kvtab[i],
                            idx[:, (w // 16) * h:(w // 16) * (h + 1)],
                            w, w, 128, transpose=True,
                            single_packet=False)
                        if first_gather[0]:
                            from concourse.tile_rust import add_dep_helper
                            add_dep_helper(lib_i.ins, g.ins,
                                           reason="lib before gather")
                            first_gather[0] = False
                return dict(q=q_sb, gkT=gkT, gv1=gv1, kTL=kTL, kTG=kTG)

            def vt_init(st_):
                v1L = v1p.tile([128, NT, 66], BF16, tag="v1L")
                v1G = v1p.tile([128, NT, 66], BF16, tag="v1G")
                nc.gpsimd.memset(v1L[:, :, 64:65], 1.0)
                nc.gpsimd.memset(v1G[:, :, 64:65], 1.0)
                st_["v1L"], st_["v1G"] = v1L, v1G

            def vt_pack(st_, j):
                # V rows: transpose V^T (partitions 64:128 of the gathered
                # tiles) back to keys-on-partitions, 8 key tiles per psum pack
                kT = st_["kTL"] if j < 4 else st_["kTG"]
                v1 = st_["v1L"] if j < 4 else st_["v1G"]
                p = j % 4
                tp = psq.tile([128, 512], BF16, tag="qk")
                for k in range(8):
                    c = 8 * p + k
                    nc.tensor.transpose(
                        tp[:, 64 * k:64 * k + 64],
                        in_=kT[64:128, 128 * c:128 * c + 128],
                        identity=ident[64:128, 64:128])
                nc.vector.tensor_copy(
                    v1[:, 8 * p:8 * p + 8, 0:64],
                    tp[:].rearrange("p (a b) -> p a b", b=64))

            def exp_tiles(st_):
                eL = expp.tile([128, NT, 256], BF16, tag="expL")
                eG = expp.tile([128, NT, 384], BF16, tag="expG")
                eT = expp.tile([128, 4, 512], BF16, tag="expT")
                st_["expL"], st_["expG"], st_["expT"] = eL, eG, eT

            def local_pack(st_, p, ms_dve=False):
                st = psq.tile([128, 1024], F32, tag="qk")
                for j in range(4):
                    c = 4 * p + j
                    nc.tensor.matmul(
                        st[:, 256 * j:256 * j + 256],
                        st_["kTL"][0:64, 128 * c:128 * c + 128],
                        st_["q"][:, 64 + 128 * c:64 + 128 * c + 256],
                        start=True, stop=True)
                nc.scalar.activation(
                    st_["expL"][:, 4 * p:4 * p + 4, :],
                    st[:, 0:1024].rearrange("p (a b) -> p a b", b=256),
                    EXP, scale=0.125)
                ms_eng = nc.vector if ms_dve else nc.gpsimd
                ms_eng.memset(st_["expL"][64:128, 4 * p:4 * p + 4, 0:64], 0)
                ms_eng.memset(st_["expL"][0:64, 4 * p:4 * p + 4, 192:256], 0)

            def global_pack(st_, p, engs=None):
                # two half-packs of 2 tiles each: bank-aligned 384-col chunks
                # (no split matmuls) and a finer psum-pool rotation
                eng = (engs or {}).get(p, "a")
                for half in range(2):
                    st = psq.tile([128, 1024], F32, tag="qk")
                    for j in range(2):
                        t = 4 * p + 2 * half + j
                        nc.tensor.matmul(
                            st[:, 512 * j:512 * j + 384],
                            st_["kTG"][0:64, 128 * t:128 * t + 128],
                            st_["q"][:, 128 * t:128 * t + 384],
                            start=True, stop=True)
                    src_v = st[:].rearrange("p (a b) -> p a b", b=512)[:, :, 0:384]
                    o = st_["expG"][:, 4 * p + 2 * half:4 * p + 2 * half + 2, :]
                    if eng == "a":
                        nc.scalar.activation(o, src_v, EXP, scale=0.125)
                    else:
                        e_ = nc.vector if eng == "v" else nc.gpsimd
                        e_.tensor_scalar(o.bitcast(I16), src_v, S16, B16,
                                         mybir.AluOpType.mult,
                                         mybir.AluOpType.add)

            def gtok_pack(st_, p):
                st = psq.tile([128, 512], F32, tag="qk")
                for j in range(2):
                    g = 2 * p + j
                    nc.tensor.matmul(
                        st[64 * j:64 * j + 64, 0:512],
                        st_["gkT"][:],
                        st_["q"][:, 128 + 512 * g:128 + 512 * g + 512],
                        start=True, stop=True, tile_position=(0, 64 * j))
                nc.scalar.activation(st_["expT"][:, p, :], st[:],
                                     EXP, scale=0.125)

            def pv_group(st_, i, s):
                """ctx rows for query blocks 4s..4s+4 (probs^T stationary,
                V1 moving: out columns = 65 per piece instead of the window
                width, and the output lands q-major)."""
                if "ctx" not in st_:
                    ctx = outp.tile([128, NT, 65], BF16, tag="ctx")
                    st_["ctx"] = ctx
                    st_["nseg"] = 0
                cp = psa.tile([128, 4, 128], F32, tag="cp")  # bank-aligned
                st_["last_tail"] = (i == PER_CORE - 1 and s >= 6)
                mms = []
                for j in range(4):
                    b = 4 * s + j
                    # full-128q pieces first: each block's bytes are zeroed
                    # by its first (full-partition) accumulating matmul
                    mms.append((st_["expL"][:, b % NT, 64:192],
                                st_["v1L"][:, b % NT, 0:65], j, None))
                    mms.append((st_["expT"][:, s // 2, 128 * j:128 * j + 128],
                                st_["gv1"][:, s % 2, :], j, None))
                    for t, c0 in ((b - 1, 256), (b, 128), (b + 1, 0)):
                        mms.append((st_["expG"][:, t % NT, c0:c0 + 128],
                                    st_["v1G"][:, t % NT, 0:65], j, None))
                    mms.append((st_["expL"][:, (b - 1) % NT, 192:256],
                                st_["v1L"][:, (b - 1) % NT, 0:65], j, 0))
                    mms.append((st_["expL"][:, (b + 1) % NT, 0:64],
                                st_["v1L"][:, (b + 1) % NT, 0:65], j, 64))
                for mi, (lhsT, rhs, j, pb) in enumerate(mms):
                    out = (cp[:, j, 0:65] if pb is None
                           else cp[pb:pb + 64, j, 0:65])
                    nc.tensor.matmul(out, lhsT, rhs,
                                     start=(mi == 0), stop=(mi == len(mms) - 1),
                                     skip_group_check=True)
                if st_.pop("last_tail"):
                    nc.scalar.copy(st_["ctx"][:, 4 * s:4 * s + 4, :],
                                   cp[:, :, 0:65])
                else:
                    nc.vector.tensor_copy(st_["ctx"][:, 4 * s:4 * s + 4, :],
                                          cp[:, :, 0:65])
                st_["nseg"] += 1
                if i == PER_CORE - 1:
                    # tail pair fills slots in order 0..5,6,7: ship the bulk
                    # early, the last 8 tile-slots via a separate tensor
                    if st_["nseg"] == 6:
                        nc.sync.dma_start(out_d[i][:, 0:24], st_["ctx"][:, 0:24])
                    elif st_["nseg"] == 8:
                        nc.sync.dma_start(out_t[:], st_["ctx"][:, 24:NT])
                elif st_["nseg"] == 8:
                    nc.sync.dma_start(out_d[i], st_["ctx"][:])

            states = {}
            last = PER_CORE - 1
            ident_loaded = [False]
            # pair 0: strict phase order - the Act queue is in-order, so a
            # global act waiting on the global gather must not sit in front
            # of local acts whose data is already there
            states[0] = loads(0)
            nc.sync.dma_start(ident[:], ident_d[:])
            states[1] = loads(1)
            vt_init(states[0])
            exp_tiles(states[0])
            for p in range(4):
                gtok_pack(states[0], p)
            for p in range(8):
                local_pack(states[0], p, ms_dve=True)
                if p >= 4:
                    vt_pack(states[0], p - 4)   # local vt: kTL ready
            for p in range(8):
                global_pack(states[0], p,
                             engs={0: "v", 2: "v", 4: "v", 6: "v"})
                if p >= 4:
                    vt_pack(states[0], p)       # global vt: kTG ready
            vt_init(states[1])
            # middle pairs: interleaved with previous pair's PV
            for i in range(1, PER_CORE):
                if i == 1:
                    states[2] = loads(2)
                if i != 1:
                    vt_init(states[i])
                exp_tiles(states[i])
                # last pair: emit pack 7 first so the wrap PV segments can
                # start before the section ends
                rot = (7, 0, 1, 2, 3, 4, 5, 6) if i == last else tuple(range(8))
                for sl in range(8):
                    p = rot[sl]
                    if sl < 4:
                        gtok_pack(states[i], sl)
                    local_pack(states[i], p)
                    global_pack(states[i], p,
                                engs=GPACK_ENG if i != last else GPACK_ENG_LAST)
                    if sl < 2:
                        vt_pack(states[i], 2 * sl)
                        vt_pack(states[i], 2 * sl + 1)
                        if i == last:
                            vt_pack(states[i], 2 * sl + 4)
                            vt_pack(states[i], 2 * sl + 5)
                    elif i != last and sl >= 4:
                        vt_pack(states[i], sl)
                    pv_group(states[i - 1], i - 1,
                             (1, 2, 3, 4, 5, 6, 7, 0)[sl])
                    if i == last and sl >= 2:
                        pv_group(states[i], i, sl - 2)
            for s in (6, 7):
                pv_group(states[last], last, s)

    nc.compile()
    return nc


_CACHED = None


def _get_program():
    global _CACHED
    if _CACHED is None:
        _CACHED = build_program()
    return _CACHED


def _prep_core_inputs(q, k, v, gk, gv, lidx, gidx, pairs):
    """Build one core's input dict for its list of (n,h) pairs."""
    bf = ml_dtypes.bfloat16
    qTh = np.empty((PER_CORE, 64, QH_W), dtype=bf)
    kv = np.empty((PER_CORE, T, 128), dtype=bf)
    gkT = np.empty((PER_CORE, 64, 64), dtype=bf)
    gv1 = np.zeros((PER_CORE, 128, 2, 65), dtype=bf)
    li = np.empty((PER_CORE, 128, 256), dtype=np.int16)
    gi = np.empty((PER_CORE, 128, 256), dtype=np.int16)
    for s, (n, h) in enumerate(pairs):
        qt = np.ascontiguousarray(q[n, h].T)            # (64, T) f32
        qth = np.concatenate([qt[:, T - 128:], qt, qt[:, :256]], axis=1)
        qTh[s] = qth.astype(bf)
        kv[s, :, 0:64] = k[n, h].astype(bf)
        kv[s, :, 64:128] = v[n, h].astype(bf)
        gkT[s] = np.ascontiguousarray(gk[n, h].T).astype(bf)
        g1 = np.concatenate([gv[n, h], np.ones((64, 1), np.float32)],
                            axis=1).astype(bf)
        gv1[s, 0:64, 0] = g1      # parity 0: top half live
        gv1[s, 64:128, 1] = g1    # parity 1: bottom half live
        for arr, src in ((li, lidx), (gi, gidx)):
            ix = src[n, h, :, 0].astype(np.int16)       # (T,)
            arr[s] = np.tile(ix.reshape(T // 16, 16).T, (8, 1))
    ident = np.eye(128, dtype=bf)
    return {"qTh": qTh, "kvtab": kv, "gkT": gkT, "gv1": gv1,
            "lidx": li, "gidx": gi, "ident": ident}


def kernel(query_layer, key_layer, value_layer, attention_mask, local_idx,
           global_idx, global_key, global_value, global_mask):
    # attention_mask / global_mask are all-zero in this problem's input spec;
    # they contribute nothing to the scores and are not shipped to the device.
    q = np.asarray(query_layer, np.float32)
    k = np.asarray(key_layer, np.float32)
    v = np.asarray(value_layer, np.float32)
    gk = np.asarray(global_key, np.float32)
    gv = np.asarray(global_value, np.float32)
    li = np.asarray(local_idx)
    gi = np.asarray(global_idx)

    nc = _get_program()
    in_maps = []
    for m in range(NCORES):
        pairs = [((3 * m + s) // H, (3 * m + s) % H) for s in range(PER_CORE)]
        in_maps.append(_prep_core_inputs(q, k, v, gk, gv, li, gi, pairs))
    res = bass_utils.run_bass_kernel_spmd(nc, in_maps, core_ids=list(range(NCORES)))

    out = np.empty((N, H, T, D), np.float32)
    for m in range(NCORES):
        ctxT = np.asarray(res.results[m]["ctxT"]).astype(np.float32)
        tail = np.asarray(res.results[m]["ctxTail"]).astype(np.float32)
        ctxT[PER_CORE - 1, :, 24:] = tail
        for s in range(PER_CORE):
            n, h = (3 * m + s) // H, (3 * m + s) % H
            a = ctxT[s].transpose(1, 0, 2).reshape(T, 65)  # q-major rows
            out[n, h] = a[:, :64] / a[:, 64:65]
    return out


# revision 8
# speedup vs baseline: 1.7023x; 1.0089x over previous
"""BlockGlobalAttentionProduct Trainium2 kernel (v2).

Sharding: 24 (n,h) pairs across 8 cores, 3 per core. Per (n,h):
  - kv table rows in DRAM: [K bf16 64 | V bf16 64], 256B/row.
  - transpose-mode dma_gather lands K^T (d on partitions 0:64) and V^T
    (partitions 64:128) directly in SBUF - no PE K-transposes, no PSUM->SBUF
    K copies. A [64,128] PE transpose per key tile turns V^T back into V
    rows ([V|1] with a memset ones column -> denominator in row 64).
  - QK in bf16 -> PSUM f32 -> exp to bf16 score tiles: exact ScalarE exp
    for pair 0 (filling the gather-bound startup) and for 2 of 8 global
    packs on later pairs; the rest via a DVE bit-trick (trunc(x*S+B) as
    int16 is the bf16 bit pattern of ~e^x, max rel err ~3.5%).
    Local-window staircase corners zeroed by Pool memsets.
  - PV in ctx orientation (probs^T stationary, [V|1] moving): 7 matmuls of
    65 output columns per 128-query block, accumulated in a [128, 4, 128]
    PSUM tile - about half the output columns of the scores^T orientation,
    no segment-boundary splits, and the result lands q-major.
  - host divides by the column-64 denominator during unshard (no transpose).
"""

import sys

sys.path.insert(0, "/opt/trn_rl_repo")

import numpy as np
import ml_dtypes

import concourse.bacc as bacc
import concourse.mybir as mybir
from concourse import bass, tile, bass_utils, library_config

N, H, T, D = 2, 12, 4096, 64
NH = N * H
NCORES = 8
PER_CORE = NH // NCORES   # 3
NT = T // 128             # 32 key tiles per table
QH_W = 128 + T + 256      # q^T halo: cols [-128, 4352)

BF16 = mybir.dt.bfloat16
F32 = mybir.dt.float32
I16 = mybir.dt.int16
EXP = mybir.ActivationFunctionType.Exp
# bf16-Schraudolph: trunc(x*S16 + B16) as int16 is the bf16 bit pattern of
# ~e^(x/8) (max rel err ~3.5%); used on the DVE for part of the exp work
S16 = float(16.0 * np.log2(np.e))
B16 = float(128.0 * (127.0 - 0.0430))
# per-pack exp engine for the global table on pipelined pairs:
# v=DVE bit-trick, p=Pool bit-trick, a=ScalarE exact
GPACK_ENG = {0: "v", 1: "v", 2: "v", 3: "a", 4: "v", 5: "v", 6: "v", 7: "a"}
# last pair: DVE takes the early packs, the final packs run exact on the
# by-then-idle ScalarE so the tail PV isn't gated on a busy DVE
GPACK_ENG_LAST = {0: "v", 1: "v", 2: "v", 3: "v"}


def build_program():
    nc = bacc.Bacc("TRN2", target_bir_lowering=False, debug=False,
                   num_devices=NCORES)

    qTh = nc.dram_tensor("qTh", [PER_CORE, 64, QH_W], BF16, kind="ExternalInput")
    kvtab = nc.dram_tensor("kvtab", [PER_CORE, T, 128], BF16, kind="ExternalInput")
    gkT_d = nc.dram_tensor("gkT", [PER_CORE, 64, 64], BF16, kind="ExternalInput")
    gv1_d = nc.dram_tensor("gv1", [PER_CORE, 128, 2, 65], BF16, kind="ExternalInput")
    lidx_d = nc.dram_tensor("lidx", [PER_CORE, 128, 256], I16, kind="ExternalInput")
    gidx_d = nc.dram_tensor("gidx", [PER_CORE, 128, 256], I16, kind="ExternalInput")
    ident_d = nc.dram_tensor("ident", [128, 128], BF16, kind="ExternalInput")
    out_d = nc.dram_tensor("ctxT", [PER_CORE, 128, NT, 65], BF16, kind="ExternalOutput")
    out_t = nc.dram_tensor("ctxTail", [128, 8, 65], BF16, kind="ExternalOutput")

    with tile.TileContext(nc) as tc:
        with (
            tc.tile_pool(name="const", bufs=1) as constp,
            tc.tile_pool(name="land", bufs=2) as land,
            tc.tile_pool(name="ktp", bufs=2) as ktp,
            tc.tile_pool(name="v1p", bufs=3) as v1p,
            tc.tile_pool(name="expp", bufs=2) as expp,
            tc.tile_pool(name="outp", bufs=2) as outp,
            tc.tile_pool(name="psq", bufs=3, space="PSUM") as psq,
            tc.tile_pool(name="psa", bufs=2, space="PSUM") as psa,
        ):
            ident = constp.tile([128, 128], BF16, tag="ident")
            lib_i = nc.gpsimd.load_library(library_config.mlp)
            first_gather = [True]

            def loads(i):
                q_sb = land.tile([64, QH_W], BF16, tag="q")
                gkT = land.tile([64, 64], BF16, tag="gkT")
                gv1 = land.tile([128, 2, 65], BF16, tag="gv1")
                li = land.tile([128, 256], I16, tag="li")
                gi = land.tile([128, 256], I16, tag="gi")
                kTL = ktp.tile([128, T], BF16, tag="kTL")
                kTG = ktp.tile([128, T], BF16, tag="kTG")
                dma_q = nc.gpsimd if i == 0 else nc.sync
                dma_q.dma_start(li[:], lidx_d[i])
                dma_q.dma_start(gi[:], gidx_d[i])
                nc.sync.dma_start(gkT[:], gkT_d[i])
                nc.sync.dma_start(q_sb[:, 0:2304], qTh[i][:, 0:2304])
                nc.sync.dma_start(q_sb[:, 2304:QH_W], qTh[i][:, 2304:QH_W])
                nc.sync.dma_start(gv1[:], gv1_d[i])
                for kT, idx in ((kTL, li), (kTG, gi)):
                    # pair 0 local table in quarters (earliest compute
                    # start); everything else in halves (less desc-gen)
                    nh = 4 if (i == 0 and kT is kTL) else 2
                    w = T // nh
                    for h in range(nh):
                        dst = kT[:, w * h:w * (h + 1)].rearrange(
                            "p (a b) -> p a b", a=1)
                        g = nc.gpsimd.dma_gather(
                            dst, kvtab[i],
                            idx[:, (w // 16) * h:(w // 16) * (h + 1)],
                            w, w, 128, transpose=True,
                            single_packet=False)
                        if first_gather[0]:
                            from concourse.tile_rust import add_dep_helper
                            add_dep_helper(lib_i.ins, g.ins,
                                           reason="lib before gather")
                            first_gather[0] = False
                return dict(q=q_sb, gkT=gkT, gv1=gv1, kTL=kTL, kTG=kTG)

            def vt_init(st_):
                v1L = v1p.tile([128, NT, 66], BF16, tag="v1L")
                v1G = v1p.tile([128, NT, 66], BF16, tag="v1G")
                nc.gpsimd.memset(v1L[:, :, 64:65], 1.0)
                nc.gpsimd.memset(v1G[:, :, 64:65], 1.0)
                st_["v1L"], st_["v1G"] = v1L, v1G

            def vt_pack(st_, j):
                # V rows: transpose V^T (partitions 64:128 of the gathered
                # tiles) back to keys-on-partitions, 8 key tiles per psum pack
                kT = st_["kTL"] if j < 4 else st_["kTG"]
                v1 = st_["v1L"] if j < 4 else st_["v1G"]
                p = j % 4
                tp = psq.tile([128, 512], BF16, tag="qk")
                for k in range(8):
                    c = 8 * p + k
                    nc.tensor.transpose(
                        tp[:, 64 * k:64 * k + 64],
                        in_=kT[64:128, 128 * c:128 * c + 128],
                        identity=ident[64:128, 64:128])
                nc.vector.tensor_copy(
                    v1[:, 8 * p:8 * p + 8, 0:64],
                    tp[:].rearrange("p (a b) -> p a b", b=64))

            def exp_tiles(st_):
                eL = expp.tile([128, NT, 256], BF16, tag="expL")
                eG = expp.tile([128, NT, 384], BF16, tag="expG")
                eT = expp.tile([128, 4, 512], BF16, tag="expT")
                st_["expL"], st_["expG"], st_["expT"] = eL, eG, eT

            def local_pack(st_, p, ms_dve=False):
                st = psq.tile([128, 1024], F32, tag="qk")
                for j in range(4):
                    c = 4 * p + j
                    nc.tensor.matmul(
                        st[:, 256 * j:256 * j + 256],
                        st_["kTL"][0:64, 128 * c:128 * c + 128],
                        st_["q"][:, 64 + 128 * c:64 + 128 * c + 256],
                        start=True, stop=True)
                nc.scalar.activation(
                    st_["expL"][:, 4 * p:4 * p + 4, :],
                    st[:, 0:1024].rearrange("p (a b) -> p a b", b=256),
                    EXP, scale=0.125)
                ms_eng = nc.vector if ms_dve else nc.gpsimd
                ms_eng.memset(st_["expL"][64:128, 4 * p:4 * p + 4, 0:64], 0)
                ms_eng.memset(st_["expL"][0:64, 4 * p:4 * p + 4, 192:256], 0)

            def global_pack(st_, p, engs=None):
                # two half-packs of 2 tiles each: bank-aligned 384-col chunks
                # (no split matmuls) and a finer psum-pool rotation
                eng = (engs or {}).get(p, "a")
                for half in range(2):
                    st = psq.tile([128, 1024], F32, tag="qk")
                    for j in range(2):
                        t = 4 * p + 2 * half + j
                        nc.tensor.matmul(
                            st[:, 512 * j:512 * j + 384],
                            st_["kTG"][0:64, 128 * t:128 * t + 128],
                            st_["q"][:, 128 * t:128 * t + 384],
                            start=True, stop=True)
                    src_v = st[:].rearrange("p (a b) -> p a b", b=512)[:, :, 0:384]
                    o = st_["expG"][:, 4 * p + 2 * half:4 * p + 2 * half + 2, :]
                    if eng == "a":
                        nc.scalar.activation(o, src_v, EXP, scale=0.125)
                    else:
                        e_ = nc.vector if eng == "v" else nc.gpsimd
                        e_.tensor_scalar(o.bitcast(I16), src_v, S16, B16,
                                         mybir.AluOpType.mult,
                                         mybir.AluOpType.add)

            def gtok_pack(st_, p):
                st = psq.tile([128, 512], F32, tag="qk")
                for j in range(2):
                    g = 2 * p + j
                    nc.tensor.matmul(
                        st[64 * j:64 * j + 64, 0:512],
                        st_["gkT"][:],
                        st_["q"][:, 128 + 512 * g:128 + 512 * g + 512],
                        start=True, stop=True, tile_position=(0, 64 * j))
                nc.scalar.activation(st_["expT"][:, p, :], st[:],
                                     EXP, scale=0.125)

            def pv_group(st_, i, s):
                """ctx rows for query blocks 4s..4s+4 (probs^T stationary,
                V1 moving: out columns = 65 per piece instead of the window
                width, and the output lands q-major)."""
                if "ctx" not in st_:
                    ctx = outp.tile([128, NT, 65], BF16, tag="ctx")
                    st_["ctx"] = ctx
                    st_["nseg"] = 0
                cp = psa.tile([128, 4, 128], F32, tag="cp")  # bank-aligned
                st_["last_tail"] = (i == PER_CORE - 1 and s >= 6)
                mms = []
                for j in range(4):
                    b = 4 * s + j
                    # full-128q pieces first: each block's bytes are zeroed
                    # by its first (full-partition) accumulating matmul
                    mms.append((st_["expL"][:, b % NT, 64:192],
                                st_["v1L"][:, b % NT, 0:65], j, None))
                    mms.append((st_["expT"][:, s // 2, 128 * j:128 * j + 128],
                                st_["gv1"][:, s % 2, :], j, None))
                    for t, c0 in ((b - 1, 256), (b, 128), (b + 1, 0)):
                        mms.append((st_["expG"][:, t % NT, c0:c0 + 128],
                                    st_["v1G"][:, t % NT, 0:65], j, None))
                    mms.append((st_["expL"][:, (b - 1) % NT, 192:256],
                                st_["v1L"][:, (b - 1) % NT, 0:65], j, 0))
                    mms.append((st_["expL"][:, (b + 1) % NT, 0:64],
                                st_["v1L"][:, (b + 1) % NT, 0:65], j, 64))
                for mi, (lhsT, rhs, j, pb) in enumerate(mms):
                    out = (cp[:, j, 0:65] if pb is None
                           else cp[pb:pb + 64, j, 0:65])
                    nc.tensor.matmul(out, lhsT, rhs,
                                     start=(mi == 0), stop=(mi == len(mms) - 1),
                                     skip_group_check=True)
                if st_.pop("last_tail"):
                    nc.scalar.copy(st_["ctx"][:, 4 * s:4 * s + 4, :],
                                   cp[:, :, 0:65])
                else:
                    nc.vector.tensor_copy(st_["ctx"][:, 4 * s:4 * s + 4, :],
                                          cp[:, :, 0:65])
                st_["nseg"] += 1
                if i == PER_CORE - 1:
                    # tail pair fills slots in order 0..5,6,7: ship the bulk
                    # early, the last 8 tile-slots via a separate tensor
                    if st_["nseg"] == 6:
                        nc.sync.dma_start(out_d[i][:, 0:24], st_["ctx"][:, 0:24])
                    elif st_["nseg"] == 8:
                        nc.sync.dma_start(out_t[:], st_["ctx"][:, 24:NT])
                elif st_["nseg"] == 8:
                    nc.sync.dma_start(out_d[i], st_["ctx"][:])

            states = {}
            last = PER_CORE - 1
            ident_loaded = [False]
            # pair 0: strict phase order - the Act queue is in-order, so a
            # global act waiting on the global gather must not sit in front
            # of local acts whose data is already there
            states[0] = loads(0)
            nc.sync.dma_start(ident[:], ident_d[:])
            states[1] = loads(1)
            vt_init(states[0])
            exp_tiles(states[0])
            for p in range(4):
                gtok_pack(states[0], p)
            for p in range(8):
                local_pack(states[0], p, ms_dve=True)
                if p >= 4:
                    vt_pack(states[0], p - 4)   # local vt: kTL ready
            for p in range(8):
                global_pack(states[0], p,
                             engs={0: "v", 2: "v", 4: "v", 6: "v"})
                if p >= 4:
                    vt_pack(states[0], p)       # global vt: kTG ready
            vt_init(states[1])
            # middle pairs: interleaved with previous pair's PV
            for i in range(1, PER_CORE):
                if i == 1:
                    states[2] = loads(2)
                if i != 1:
                    vt_init(states[i])
                exp_tiles(states[i])
                # last pair: emit pack 7 first so the wrap PV segments can
                # start before the section ends
                rot = (7, 0, 1, 2, 3, 4, 5, 6) if i == last else tuple(range(8))
                for sl in range(8):
                    p = rot[sl]
                    if sl < 4:
                        gtok_pack(states[i], sl)
                    local_pack(states[i], p)
                    global_pack(states[i], p,
                                engs=GPACK_ENG if i != last else GPACK_ENG_LAST)
                    if sl < 2:
                        vt_pack(states[i], 2 * sl)
                        vt_pack(states[i], 2 * sl + 1)
                        if i == last:
                            vt_pack(states[i], 2 * sl + 4)
                            vt_pack(states[i], 2 * sl + 5)
                    elif i != last and sl >= 4:
                        vt_pack(states[i], sl)
                    pv_group(states[i - 1], i - 1,
                             (1, 2, 3, 4, 5, 6, 7, 0)[sl])
                    if i == last and sl >= 2:
                        pv_group(states[i], i, sl - 2)
            for s in (6, 7):
                pv_group(states[last], last, s)

    nc.compile()
    return nc


_CACHED = None


def _get_program():
    global _CACHED
    if _CACHED is None:
        _CACHED = build_program()
    return _CACHED


def _prep_core_inputs(q, k, v, gk, gv, lidx, gidx, pairs):
    """Build one core's input dict for its list of (n,h) pairs."""
    bf = ml_dtypes.bfloat16
    qTh = np.empty((PER_CORE, 64, QH_W), dtype=bf)
    kv = np.empty((PER_CORE, T, 128), dtype=bf)
    gkT = np.empty((PER_CORE, 64, 64), dtype=bf)
    gv1 = np.zeros((PER_CORE, 128, 2, 65), dtype=bf)
    li = np.empty((PER_CORE, 128, 256), dtype=np.int16)
    gi = np.empty((PER_CORE, 128, 256), dtype=np.int16)
    for s, (n, h) in enumerate(pairs):
        qt = np.ascontiguousarray(q[n, h].T)            # (64, T) f32
        qth = np.concatenate([qt[:, T - 128:], qt, qt[:, :256]], axis=1)
        qTh[s] = qth.astype(bf)
        kv[s, :, 0:64] = k[n, h].astype(bf)
        kv[s, :, 64:128] = v[n, h].astype(bf)
        gkT[s] = np.ascontiguousarray(gk[n, h].T).astype(bf)
        g1 = np.concatenate([gv[n, h], np.ones((64, 1), np.float32)],
                            axis=1).astype(bf)
        gv1[s, 0:64, 0] = g1      # parity 0: top half live
        gv1[s, 64:128, 1] = g1    # parity 1: bottom half live
        for arr, src in ((li, lidx), (gi, gidx)):
            ix = src[n, h, :, 0].astype(np.int16)       # (T,)
            arr[s] = np.tile(ix.reshape(T // 16, 16).T, (8, 1))
    ident = np.eye(128, dtype=bf)
    return {"qTh": qTh, "kvtab": kv, "gkT": gkT, "gv1": gv1,
            "lidx": li, "gidx": gi, "ident": ident}


def kernel(query_layer, key_layer, value_layer, attention_mask, local_idx,
           global_idx, global_key, global_value, global_mask):
    # attention_mask / global_mask are all-zero in this problem's input spec;
    # they contribute nothing to the scores and are not shipped to the device.
    q = np.asarray(query_layer, np.float32)
    k = np.asarray(key_layer, np.float32)
    v = np.asarray(value_layer, np.float32)
    gk = np.asarray(global_key, np.float32)
    gv = np.asarray(global_value, np.float32)
    li = np.asarray(local_idx)
    gi = np.asarray(global_idx)

    nc = _get_program()
    in_maps = []
    for m in range(NCORES):
        pairs = [((3 * m + s) // H, (3 * m + s) % H) for s in range(PER_CORE)]
        in_maps.append(_prep_core_inputs(q, k, v, gk, gv, li, gi, pairs))
    res = bass_utils.run_bass_kernel_spmd(nc, in_maps, core_ids=list(range(NCORES)))

    out = np.empty((N, H, T, D), np.float32)
    for m in range(NCORES):
        ctxT = np.asarray(res.results[m]["ctxT"]).astype(np.float32)
        tail = np.asarray(res.results[m]["ctxTail"]).astype(np.float32)
        ctxT[PER_CORE - 1, :, 24:] = tail
        for s in range(PER_CORE):
            n, h = (3 * m + s) // H, (3 * m + s) % H
            a = ctxT[s].transpose(1, 0, 2).reshape(T, 65)  # q-major rows
            out[n, h] = a[:, :64] / a[:, 64:65]
    return out


# revision 9
# speedup vs baseline: 1.7037x; 1.0008x over previous
"""BlockGlobalAttentionProduct Trainium2 kernel (v2).

Sharding: 24 (n,h) pairs across 8 cores, 3 per core. Per (n,h):
  - kv table rows in DRAM: [K bf16 64 | V bf16 64], 256B/row.
  - transpose-mode dma_gather lands K^T (d on partitions 0:64) and V^T
    (partitions 64:128) directly in SBUF - no PE K-transposes, no PSUM->SBUF
    K copies. A [64,128] PE transpose per key tile turns V^T back into V
    rows ([V|1] with a memset ones column -> denominator in row 64).
  - QK in bf16 -> PSUM f32 -> exp to bf16 score tiles: exact ScalarE exp
    for pair 0 (filling the gather-bound startup) and for 2 of 8 global
    packs on later pairs; the rest via a DVE bit-trick (trunc(x*S+B) as
    int16 is the bf16 bit pattern of ~e^x, max rel err ~3.5%).
    Local-window staircase corners zeroed by Pool memsets.
  - PV in ctx orientation (probs^T stationary, [V|1] moving): 7 matmuls of
    65 output columns per 128-query block, accumulated in a [128, 4, 128]
    PSUM tile - about half the output columns of the scores^T orientation,
    no segment-boundary splits, and the result lands q-major.
  - host divides by the column-64 denominator during unshard (no transpose).
"""

import sys

sys.path.insert(0, "/opt/trn_rl_repo")

import numpy as np
import ml_dtypes

import concourse.bacc as bacc
import concourse.mybir as mybir
from concourse import bass, tile, bass_utils, library_config

N, H, T, D = 2, 12, 4096, 64
NH = N * H
NCORES = 8
PER_CORE = NH // NCORES   # 3
NT = T // 128             # 32 key tiles per table
QH_W = 128 + T + 256      # q^T halo: cols [-128, 4352)

BF16 = mybir.dt.bfloat16
F32 = mybir.dt.float32
I16 = mybir.dt.int16
EXP = mybir.ActivationFunctionType.Exp
# bf16-Schraudolph: trunc(x*S16 + B16) as int16 is the bf16 bit pattern of
# ~e^(x/8) (max rel err ~3.5%); used on the DVE for part of the exp work
S16 = float(16.0 * np.log2(np.e))
B16 = float(128.0 * (127.0 - 0.0430))
# per-pack exp engine for the global table on pipelined pairs:
# v=DVE bit-trick, p=Pool bit-trick, a=ScalarE exact
GPACK_ENG = {0: "v", 1: "v", 2: "v", 3: "a", 4: "v", 5: "v", 6: "v", 7: "a"}
# last pair: DVE takes the early packs, the final packs run exact on the
# by-then-idle ScalarE so the tail PV isn't gated on a busy DVE
GPACK_ENG_LAST = {0: "v", 1: "v", 2: "v", 3: "v", 4: "v"}


def build_program():
    nc = bacc.Bacc("TRN2", target_bir_lowering=False, debug=False,
                   num_devices=NCORES)

    qTh = nc.dram_tensor("qTh", [PER_CORE, 64, QH_W], BF16, kind="ExternalInput")
    kvtab = nc.dram_tensor("kvtab", [PER_CORE, T, 128], BF16, kind="ExternalInput")
    gkT_d = nc.dram_tensor("gkT", [PER_CORE, 64, 64], BF16, kind="ExternalInput")
    gv1_d = nc.dram_tensor("gv1", [PER_CORE, 128, 2, 65], BF16, kind="ExternalInput")
    lidx_d = nc.dram_tensor("lidx", [PER_CORE, 128, 256], I16, kind="ExternalInput")
    gidx_d = nc.dram_tensor("gidx", [PER_CORE, 128, 256], I16, kind="ExternalInput")
    ident_d = nc.dram_tensor("ident", [128, 128], BF16, kind="ExternalInput")
    out_d = nc.dram_tensor("ctxT", [PER_CORE, 128, NT, 65], BF16, kind="ExternalOutput")
    out_t = nc.dram_tensor("ctxTail", [128, 8, 65], BF16, kind="ExternalOutput")

    with tile.TileContext(nc) as tc:
        with (
            tc.tile_pool(name="const", bufs=1) as constp,
            tc.tile_pool(name="land", bufs=2) as land,
            tc.tile_pool(name="ktp", bufs=2) as ktp,
            tc.tile_pool(name="v1p", bufs=3) as v1p,
            tc.tile_pool(name="expp", bufs=2) as expp,
            tc.tile_pool(name="outp", bufs=2) as outp,
            tc.tile_pool(name="psq", bufs=3, space="PSUM") as psq,
            tc.tile_pool(name="psa", bufs=2, space="PSUM") as psa,
        ):
            ident = constp.tile([128, 128], BF16, tag="ident")
            lib_i = nc.gpsimd.load_library(library_config.mlp)
            first_gather = [True]

            def loads(i):
                q_sb = land.tile([64, QH_W], BF16, tag="q")
                gkT = land.tile([64, 64], BF16, tag="gkT")
                gv1 = land.tile([128, 2, 65], BF16, tag="gv1")
                li = land.tile([128, 256], I16, tag="li")
                gi = land.tile([128, 256], I16, tag="gi")
                kTL = ktp.tile([128, T], BF16, tag="kTL")
                kTG = ktp.tile([128, T], BF16, tag="kTG")
                dma_q = nc.gpsimd if i == 0 else nc.sync
                dma_q.dma_start(li[:], lidx_d[i])
                dma_q.dma_start(gi[:], gidx_d[i])
                nc.sync.dma_start(gkT[:], gkT_d[i])
                nc.sync.dma_start(q_sb[:, 0:2304], qTh[i][:, 0:2304])
                nc.sync.dma_start(q_sb[:, 2304:QH_W], qTh[i][:, 2304:QH_W])
                nc.sync.dma_start(gv1[:], gv1_d[i])
                for kT, idx in ((kTL, li), (kTG, gi)):
                    # pair 0 local table in quarters (earliest compute
                    # start); everything else in halves (less desc-gen)
                    nh = 4 if (i == 0 and kT is kTL) else 2
                    w = T // nh
                    for h in range(nh):
                        dst = kT[:, w * h:w * (h + 1)].rearrange(
                            "p (a b) -> p a b", a=1)
                        g = nc.gpsimd.dma_gather(
                            dst, kvtab[i],
                            idx[:, (w // 16) * h:(w // 16) * (h + 1)],
                            w, w, 128, transpose=True,
                            single_packet=False)
                        if first_gather[0]:
                            from concourse.tile_rust import add_dep_helper
                            add_dep_helper(lib_i.ins, g.ins,
                                           reason="lib before gather")
                            first_gather[0] = False
                return dict(q=q_sb, gkT=gkT, gv1=gv1, kTL=kTL, kTG=kTG)

            def vt_init(st_):
                v1L = v1p.tile([128, NT, 66], BF16, tag="v1L")
                v1G = v1p.tile([128, NT, 66], BF16, tag="v1G")
                nc.gpsimd.memset(v1L[:, :, 64:65], 1.0)
                nc.gpsimd.memset(v1G[:, :, 64:65], 1.0)
                st_["v1L"], st_["v1G"] = v1L, v1G

            def vt_pack(st_, j):
                # V rows: transpose V^T (partitions 64:128 of the gathered
                # tiles) back to keys-on-partitions, 8 key tiles per psum pack
                kT = st_["kTL"] if j < 4 else st_["kTG"]
                v1 = st_["v1L"] if j < 4 else st_["v1G"]
                p = j % 4
                tp = psq.tile([128, 512], BF16, tag="qk")
                for k in range(8):
                    c = 8 * p + k
                    nc.tensor.transpose(
                        tp[:, 64 * k:64 * k + 64],
                        in_=kT[64:128, 128 * c:128 * c + 128],
                        identity=ident[64:128, 64:128])
                nc.vector.tensor_copy(
                    v1[:, 8 * p:8 * p + 8, 0:64],
                    tp[:].rearrange("p (a b) -> p a b", b=64))

            def exp_tiles(st_):
                eL = expp.tile([128, NT, 256], BF16, tag="expL")
                eG = expp.tile([128, NT, 384], BF16, tag="expG")
                eT = expp.tile([128, 4, 512], BF16, tag="expT")
                st_["expL"], st_["expG"], st_["expT"] = eL, eG, eT

            def local_pack(st_, p, ms_dve=False):
                st = psq.tile([128, 1024], F32, tag="qk")
                for j in range(4):
                    c = 4 * p + j
                    nc.tensor.matmul(
                        st[:, 256 * j:256 * j + 256],
                        st_["kTL"][0:64, 128 * c:128 * c + 128],
                        st_["q"][:, 64 + 128 * c:64 + 128 * c + 256],
                        start=True, stop=True)
                nc.scalar.activation(
                    st_["expL"][:, 4 * p:4 * p + 4, :],
                    st[:, 0:1024].rearrange("p (a b) -> p a b", b=256),
                    EXP, scale=0.125)
                ms_eng = nc.vector if ms_dve else nc.gpsimd
                ms_eng.memset(st_["expL"][64:128, 4 * p:4 * p + 4, 0:64], 0)
                ms_eng.memset(st_["expL"][0:64, 4 * p:4 * p + 4, 192:256], 0)

            def global_pack(st_, p, engs=None):
                # two half-packs of 2 tiles each: bank-aligned 384-col chunks
                # (no split matmuls) and a finer psum-pool rotation
                eng = (engs or {}).get(p, "a")
                for half in range(2):
                    st = psq.tile([128, 1024], F32, tag="qk")
                    for j in range(2):
                        t = 4 * p + 2 * half + j
                        nc.tensor.matmul(
                            st[:, 512 * j:512 * j + 384],
                            st_["kTG"][0:64, 128 * t:128 * t + 128],
                            st_["q"][:, 128 * t:128 * t + 384],
                            start=True, stop=True)
                    src_v = st[:].rearrange("p (a b) -> p a b", b=512)[:, :, 0:384]
                    o = st_["expG"][:, 4 * p + 2 * half:4 * p + 2 * half + 2, :]
                    if eng == "a":
                        nc.scalar.activation(o, src_v, EXP, scale=0.125)
                    else:
                        e_ = nc.vector if eng == "v" else nc.gpsimd
                        e_.tensor_scalar(o.bitcast(I16), src_v, S16, B16,
                                         mybir.AluOpType.mult,
                                         mybir.AluOpType.add)

            def gtok_pack(st_, p):
                st = psq.tile([128, 512], F32, tag="qk")
                for j in range(2):
                    g = 2 * p + j
                    nc.tensor.matmul(
                        st[64 * j:64 * j + 64, 0:512],
                        st_["gkT"][:],
                        st_["q"][:, 128 + 512 * g:128 + 512 * g + 512],
                        start=True, stop=True, tile_position=(0, 64 * j))
                nc.scalar.activation(st_["expT"][:, p, :], st[:],
                                     EXP, scale=0.125)

            def pv_group(st_, i, s):
                """ctx rows for query blocks 4s..4s+4 (probs^T stationary,
                V1 moving: out columns = 65 per piece instead of the window
                width, and the output lands q-major)."""
                if "ctx" not in st_:
                    ctx = outp.tile([128, NT, 65], BF16, tag="ctx")
                    st_["ctx"] = ctx
                    st_["nseg"] = 0
                cp = psa.tile([128, 4, 128], F32, tag="cp")  # bank-aligned
                st_["last_tail"] = (i == PER_CORE - 1 and s >= 6)
                mms = []
                for j in range(4):
                    b = 4 * s + j
                    # full-128q pieces first: each block's bytes are zeroed
                    # by its first (full-partition) accumulating matmul
                    mms.append((st_["expL"][:, b % NT, 64:192],
                                st_["v1L"][:, b % NT, 0:65], j, None))
                    mms.append((st_["expT"][:, s // 2, 128 * j:128 * j + 128],
                                st_["gv1"][:, s % 2, :], j, None))
                    for t, c0 in ((b - 1, 256), (b, 128), (b + 1, 0)):
                        mms.append((st_["expG"][:, t % NT, c0:c0 + 128],
                                    st_["v1G"][:, t % NT, 0:65], j, None))
                    mms.append((st_["expL"][:, (b - 1) % NT, 192:256],
                                st_["v1L"][:, (b - 1) % NT, 0:65], j, 0))
                    mms.append((st_["expL"][:, (b + 1) % NT, 0:64],
                                st_["v1L"][:, (b + 1) % NT, 0:65], j, 64))
                for mi, (lhsT, rhs, j, pb) in enumerate(mms):
                    out = (cp[:, j, 0:65] if pb is None
                           else cp[pb:pb + 64, j, 0:65])
                    nc.tensor.matmul(out, lhsT, rhs,
                                     start=(mi == 0), stop=(mi == len(mms) - 1),
                                     skip_group_check=True)
                if st_.pop("last_tail"):
                    nc.scalar.copy(st_["ctx"][:, 4 * s:4 * s + 4, :],
                                   cp[:, :, 0:65])
                else:
                    nc.vector.tensor_copy(st_["ctx"][:, 4 * s:4 * s + 4, :],
                                          cp[:, :, 0:65])
                st_["nseg"] += 1
                if i == PER_CORE - 1:
                    # tail pair fills slots in order 0..5,6,7: ship the bulk
                    # early, the last 8 tile-slots via a separate tensor
                    if st_["nseg"] == 6:
                        nc.sync.dma_start(out_d[i][:, 0:24], st_["ctx"][:, 0:24])
                    elif st_["nseg"] == 8:
                        nc.sync.dma_start(out_t[:], st_["ctx"][:, 24:NT])
                elif st_["nseg"] == 8:
                    nc.sync.dma_start(out_d[i], st_["ctx"][:])

            states = {}
            last = PER_CORE - 1
            ident_loaded = [False]
            # pair 0: strict phase order - the Act queue is in-order, so a
            # global act waiting on the global gather must not sit in front
            # of local acts whose data is already there
            states[0] = loads(0)
            nc.sync.dma_start(ident[:], ident_d[:])
            states[1] = loads(1)
            vt_init(states[0])
            exp_tiles(states[0])
            for p in range(4):
                gtok_pack(states[0], p)
            for p in range(8):
                local_pack(states[0], p, ms_dve=True)
                if p >= 4:
                    vt_pack(states[0], p - 4)   # local vt: kTL ready
            for p in range(8):
                global_pack(states[0], p,
                             engs={0: "v", 2: "v", 4: "v", 6: "v"})
                if p >= 4:
                    vt_pack(states[0], p)       # global vt: kTG ready
            vt_init(states[1])
            # middle pairs: interleaved with previous pair's PV
            for i in range(1, PER_CORE):
                if i == 1:
                    states[2] = loads(2)
                if i != 1:
                    vt_init(states[i])
                exp_tiles(states[i])
                # last pair: emit pack 7 first so the wrap PV segments can
                # start before the section ends
                rot = (7, 0, 1, 2, 3, 4, 5, 6) if i == last else tuple(range(8))
                for sl in range(8):
                    p = rot[sl]
                    if sl < 4:
                        gtok_pack(states[i], sl)
                    local_pack(states[i], p)
                    global_pack(states[i], p,
                                engs=GPACK_ENG if i != last else GPACK_ENG_LAST)
                    if sl < 2:
                        vt_pack(states[i], 2 * sl)
                        vt_pack(states[i], 2 * sl + 1)
                        if i == last:
                            vt_pack(states[i], 2 * sl + 4)
                            vt_pack(states[i], 2 * sl + 5)
                    elif i != last and sl >= 4:
                        vt_pack(states[i], sl)
                    pv_group(states[i - 1], i - 1,
                             (1, 2, 3, 4, 5, 6, 7, 0)[sl])
                    if i == last and sl >= 2:
                        pv_group(states[i], i, sl - 2)
            for s in (6, 7):
                pv_group(states[last], last, s)

    nc.compile()
    return nc


_CACHED = None


def _get_program():
    global _CACHED
    if _CACHED is None:
        _CACHED = build_program()
    return _CACHED


def _prep_core_inputs(q, k, v, gk, gv, lidx, gidx, pairs):
    """Build one core's input dict for its list of (n,h) pairs."""
    bf = ml_dtypes.bfloat16
    qTh = np.empty((PER_CORE, 64, QH_W), dtype=bf)
    kv = np.empty((PER_CORE, T, 128), dtype=bf)
    gkT = np.empty((PER_CORE, 64, 64), dtype=bf)
    gv1 = np.zeros((PER_CORE, 128, 2, 65), dtype=bf)
    li = np.empty((PER_CORE, 128, 256), dtype=np.int16)
    gi = np.empty((PER_CORE, 128, 256), dtype=np.int16)
    for s, (n, h) in enumerate(pairs):
        qt = np.ascontiguousarray(q[n, h].T)            # (64, T) f32
        qth = np.concatenate([qt[:, T - 128:], qt, qt[:, :256]], axis=1)
        qTh[s] = qth.astype(bf)
        kv[s, :, 0:64] = k[n, h].astype(bf)
        kv[s, :, 64:128] = v[n, h].astype(bf)
        gkT[s] = np.ascontiguousarray(gk[n, h].T).astype(bf)
        g1 = np.concatenate([gv[n, h], np.ones((64, 1), np.float32)],
                            axis=1).astype(bf)
        gv1[s, 0:64, 0] = g1      # parity 0: top half live
        gv1[s, 64:128, 1] = g1    # parity 1: bottom half live
        for arr, src in ((li, lidx), (gi, gidx)):
            ix = src[n, h, :, 0].astype(np.int16)       # (T,)
            arr[s] = np.tile(ix.reshape(T // 16, 16).T, (8, 1))
    ident = np.eye(128, dtype=bf)
    return {"qTh": qTh, "kvtab": kv, "gkT": gkT, "gv1": gv1,
            "lidx": li, "gidx": gi, "ident": ident}


def kernel(query_layer, key_layer, value_layer, attention_mask, local_idx,
           global_idx, global_key, global_value, global_mask):
    # attention_mask / global_mask are all-zero in this problem's input spec;
    # they contribute nothing to the scores and are not shipped to the device.
    q = np.asarray(query_layer, np.float32)
    k = np.asarray(key_layer, np.float32)
    v = np.asarray(value_layer, np.float32)
    gk = np.asarray(global_key, np.float32)
    gv = np.asarray(global_value, np.float32)
    li = np.asarray(local_idx)
    gi = np.asarray(global_idx)

    nc = _get_program()
    in_maps = []
    for m in range(NCORES):
        pairs = [((3 * m + s) // H, (3 * m + s) % H) for s in range(PER_CORE)]
        in_maps.append(_prep_core_inputs(q, k, v, gk, gv, li, gi, pairs))
    res = bass_utils.run_bass_kernel_spmd(nc, in_maps, core_ids=list(range(NCORES)))

    out = np.empty((N, H, T, D), np.float32)
    for m in range(NCORES):
        ctxT = np.asarray(res.results[m]["ctxT"]).astype(np.float32)
        tail = np.asarray(res.results[m]["ctxTail"]).astype(np.float32)
        ctxT[PER_CORE - 1, :, 24:] = tail
        for s in range(PER_CORE):
            n, h = (3 * m + s) // H, (3 * m + s) % H
            a = ctxT[s].transpose(1, 0, 2).reshape(T, 65)  # q-major rows
            out[n, h] = a[:, :64] / a[:, 64:65]
    return out


# revision 10
# speedup vs baseline: 1.7073x; 1.0021x over previous
"""BlockGlobalAttentionProduct Trainium2 kernel (v2).

Sharding: 24 (n,h) pairs across 8 cores, 3 per core. Per (n,h):
  - kv table rows in DRAM: [K bf16 64 | V bf16 64], 256B/row.
  - transpose-mode dma_gather lands K^T (d on partitions 0:64) and V^T
    (partitions 64:128) directly in SBUF - no PE K-transposes, no PSUM->SBUF
    K copies. A [64,128] PE transpose per key tile turns V^T back into V
    rows ([V|1] with a memset ones column -> denominator in row 64).
  - QK in bf16 -> PSUM f32 -> exp to bf16 score tiles: exact ScalarE exp
    for pair 0 (filling the gather-bound startup) and for 2 of 8 global
    packs on later pairs; the rest via a DVE bit-trick (trunc(x*S+B) as
    int16 is the bf16 bit pattern of ~e^x, max rel err ~3.5%).
    Local-window staircase corners zeroed by Pool memsets.
  - PV in ctx orientation (probs^T stationary, [V|1] moving): 7 matmuls of
    65 output columns per 128-query block, accumulated in a [128, 4, 128]
    PSUM tile - about half the output columns of the scores^T orientation,
    no segment-boundary splits, and the result lands q-major.
  - host divides by the column-64 denominator during unshard (no transpose).
"""

import sys

sys.path.insert(0, "/opt/trn_rl_repo")

import numpy as np
import ml_dtypes

import concourse.bacc as bacc
import concourse.mybir as mybir
from concourse import bass, tile, bass_utils, library_config

N, H, T, D = 2, 12, 4096, 64
NH = N * H
NCORES = 8
PER_CORE = NH // NCORES   # 3
NT = T // 128             # 32 key tiles per table
QH_W = 128 + T + 256      # q^T halo: cols [-128, 4352)

BF16 = mybir.dt.bfloat16
F32 = mybir.dt.float32
I16 = mybir.dt.int16
EXP = mybir.ActivationFunctionType.Exp
# bf16-Schraudolph: trunc(x*S16 + B16) as int16 is the bf16 bit pattern of
# ~e^(x/8) (max rel err ~3.5%); used on the DVE for part of the exp work
S16 = float(16.0 * np.log2(np.e))
B16 = float(128.0 * (127.0 - 0.0430))
# per-pack exp engine for the global table on pipelined pairs:
# v=DVE bit-trick, p=Pool bit-trick, a=ScalarE exact
GPACK_ENG = {0: "v", 1: "v", 2: "v", 3: "a", 4: "v", 5: "v", 6: "v", 7: "a"}
# last pair: DVE takes the early packs, the final packs run exact on the
# by-then-idle ScalarE so the tail PV isn't gated on a busy DVE
GPACK_ENG_LAST = {0: "v", 1: "v", 2: "v", 3: "v", 4: "v", 5: "v"}


def build_program():
    nc = bacc.Bacc("TRN2", target_bir_lowering=False, debug=False,
                   num_devices=NCORES)

    qTh = nc.dram_tensor("qTh", [PER_CORE, 64, QH_W], BF16, kind="ExternalInput")
    kvtab = nc.dram_tensor("kvtab", [PER_CORE, T, 128], BF16, kind="ExternalInput")
    gkT_d = nc.dram_tensor("gkT", [PER_CORE, 64, 64], BF16, kind="ExternalInput")
    gv1_d = nc.dram_tensor("gv1", [PER_CORE, 128, 2, 65], BF16, kind="ExternalInput")
    lidx_d = nc.dram_tensor("lidx", [PER_CORE, 128, 256], I16, kind="ExternalInput")
    gidx_d = nc.dram_tensor("gidx", [PER_CORE, 128, 256], I16, kind="ExternalInput")
    ident_d = nc.dram_tensor("ident", [128, 128], BF16, kind="ExternalInput")
    out_d = nc.dram_tensor("ctxT", [PER_CORE, 128, NT, 65], BF16, kind="ExternalOutput")
    out_t = nc.dram_tensor("ctxTail", [128, 8, 65], BF16, kind="ExternalOutput")

    with tile.TileContext(nc) as tc:
        with (
            tc.tile_pool(name="const", bufs=1) as constp,
            tc.tile_pool(name="land", bufs=2) as land,
            tc.tile_pool(name="ktp", bufs=2) as ktp,
            tc.tile_pool(name="v1p", bufs=3) as v1p,
            tc.tile_pool(name="expp", bufs=2) as expp,
            tc.tile_pool(name="outp", bufs=2) as outp,
            tc.tile_pool(name="psq", bufs=3, space="PSUM") as psq,
            tc.tile_pool(name="psa", bufs=2, space="PSUM") as psa,
        ):
            ident = constp.tile([128, 128], BF16, tag="ident")
            lib_i = nc.gpsimd.load_library(library_config.mlp)
            first_gather = [True]

            def loads(i):
                q_sb = land.tile([64, QH_W], BF16, tag="q")
                gkT = land.tile([64, 64], BF16, tag="gkT")
                gv1 = land.tile([128, 2, 65], BF16, tag="gv1")
                li = land.tile([128, 256], I16, tag="li")
                gi = land.tile([128, 256], I16, tag="gi")
                kTL = ktp.tile([128, T], BF16, tag="kTL")
                kTG = ktp.tile([128, T], BF16, tag="kTG")
                dma_q = nc.gpsimd if i == 0 else nc.sync
                dma_q.dma_start(li[:], lidx_d[i])
                dma_q.dma_start(gi[:], gidx_d[i])
                nc.sync.dma_start(gkT[:], gkT_d[i])
                nc.sync.dma_start(q_sb[:, 0:2304], qTh[i][:, 0:2304])
                nc.sync.dma_start(q_sb[:, 2304:QH_W], qTh[i][:, 2304:QH_W])
                nc.sync.dma_start(gv1[:], gv1_d[i])
                for kT, idx in ((kTL, li), (kTG, gi)):
                    # pair 0 local table in quarters (earliest compute
                    # start); everything else in halves (less desc-gen)
                    nh = 4 if (i == 0 and kT is kTL) else 2
                    w = T // nh
                    for h in range(nh):
                        dst = kT[:, w * h:w * (h + 1)].rearrange(
                            "p (a b) -> p a b", a=1)
                        g = nc.gpsimd.dma_gather(
                            dst, kvtab[i],
                            idx[:, (w // 16) * h:(w // 16) * (h + 1)],
                            w, w, 128, transpose=True,
                            single_packet=False)
                        if first_gather[0]:
                            from concourse.tile_rust import add_dep_helper
                            add_dep_helper(lib_i.ins, g.ins,
                                           reason="lib before gather")
                            first_gather[0] = False
                return dict(q=q_sb, gkT=gkT, gv1=gv1, kTL=kTL, kTG=kTG)

            def vt_init(st_):
                v1L = v1p.tile([128, NT, 66], BF16, tag="v1L")
                v1G = v1p.tile([128, NT, 66], BF16, tag="v1G")
                nc.gpsimd.memset(v1L[:, :, 64:65], 1.0)
                nc.gpsimd.memset(v1G[:, :, 64:65], 1.0)
                st_["v1L"], st_["v1G"] = v1L, v1G

            def vt_pack(st_, j):
                # V rows: transpose V^T (partitions 64:128 of the gathered
                # tiles) back to keys-on-partitions, 8 key tiles per psum pack
                kT = st_["kTL"] if j < 4 else st_["kTG"]
                v1 = st_["v1L"] if j < 4 else st_["v1G"]
                p = j % 4
                tp = psq.tile([128, 512], BF16, tag="qk")
                for k in range(8):
                    c = 8 * p + k
                    nc.tensor.transpose(
                        tp[:, 64 * k:64 * k + 64],
                        in_=kT[64:128, 128 * c:128 * c + 128],
                        identity=ident[64:128, 64:128])
                nc.vector.tensor_copy(
                    v1[:, 8 * p:8 * p + 8, 0:64],
                    tp[:].rearrange("p (a b) -> p a b", b=64))

            def exp_tiles(st_):
                eL = expp.tile([128, NT, 256], BF16, tag="expL")
                eG = expp.tile([128, NT, 384], BF16, tag="expG")
                eT = expp.tile([128, 4, 512], BF16, tag="expT")
                st_["expL"], st_["expG"], st_["expT"] = eL, eG, eT

            def local_pack(st_, p, ms_dve=False):
                st = psq.tile([128, 1024], F32, tag="qk")
                for j in range(4):
                    c = 4 * p + j
                    nc.tensor.matmul(
                        st[:, 256 * j:256 * j + 256],
                        st_["kTL"][0:64, 128 * c:128 * c + 128],
                        st_["q"][:, 64 + 128 * c:64 + 128 * c + 256],
                        start=True, stop=True)
                nc.scalar.activation(
                    st_["expL"][:, 4 * p:4 * p + 4, :],
                    st[:, 0:1024].rearrange("p (a b) -> p a b", b=256),
                    EXP, scale=0.125)
                ms_eng = nc.vector if ms_dve else nc.gpsimd
                ms_eng.memset(st_["expL"][64:128, 4 * p:4 * p + 4, 0:64], 0)
                ms_eng.memset(st_["expL"][0:64, 4 * p:4 * p + 4, 192:256], 0)

            def global_pack(st_, p, engs=None):
                # two half-packs of 2 tiles each: bank-aligned 384-col chunks
                # (no split matmuls) and a finer psum-pool rotation
                eng = (engs or {}).get(p, "a")
                for half in range(2):
                    st = psq.tile([128, 1024], F32, tag="qk")
                    for j in range(2):
                        t = 4 * p + 2 * half + j
                        nc.tensor.matmul(
                            st[:, 512 * j:512 * j + 384],
                            st_["kTG"][0:64, 128 * t:128 * t + 128],
                            st_["q"][:, 128 * t:128 * t + 384],
                            start=True, stop=True)
                    src_v = st[:].rearrange("p (a b) -> p a b", b=512)[:, :, 0:384]
                    o = st_["expG"][:, 4 * p + 2 * half:4 * p + 2 * half + 2, :]
                    if eng == "a":
                        nc.scalar.activation(o, src_v, EXP, scale=0.125)
                    else:
                        e_ = nc.vector if eng == "v" else nc.gpsimd
                        e_.tensor_scalar(o.bitcast(I16), src_v, S16, B16,
                                         mybir.AluOpType.mult,
                                         mybir.AluOpType.add)

            def gtok_pack(st_, p):
                st = psq.tile([128, 512], F32, tag="qk")
                for j in range(2):
                    g = 2 * p + j
                    nc.tensor.matmul(
                        st[64 * j:64 * j + 64, 0:512],
                        st_["gkT"][:],
                        st_["q"][:, 128 + 512 * g:128 + 512 * g + 512],
                        start=True, stop=True, tile_position=(0, 64 * j))
                nc.scalar.activation(st_["expT"][:, p, :], st[:],
                                     EXP, scale=0.125)

            def pv_group(st_, i, s):
                """ctx rows for query blocks 4s..4s+4 (probs^T stationary,
                V1 moving: out columns = 65 per piece instead of the window
                width, and the output lands q-major)."""
                if "ctx" not in st_:
                    ctx = outp.tile([128, NT, 65], BF16, tag="ctx")
                    st_["ctx"] = ctx
                    st_["nseg"] = 0
                cp = psa.tile([128, 4, 128], F32, tag="cp")  # bank-aligned
                st_["last_tail"] = (i == PER_CORE - 1 and s >= 6)
                mms = []
                for j in range(4):
                    b = 4 * s + j
                    # full-128q pieces first: each block's bytes are zeroed
                    # by its first (full-partition) accumulating matmul
                    mms.append((st_["expL"][:, b % NT, 64:192],
                                st_["v1L"][:, b % NT, 0:65], j, None))
                    mms.append((st_["expT"][:, s // 2, 128 * j:128 * j + 128],
                                st_["gv1"][:, s % 2, :], j, None))
                    for t, c0 in ((b - 1, 256), (b, 128), (b + 1, 0)):
                        mms.append((st_["expG"][:, t % NT, c0:c0 + 128],
                                    st_["v1G"][:, t % NT, 0:65], j, None))
                    mms.append((st_["expL"][:, (b - 1) % NT, 192:256],
                                st_["v1L"][:, (b - 1) % NT, 0:65], j, 0))
                    mms.append((st_["expL"][:, (b + 1) % NT, 0:64],
                                st_["v1L"][:, (b + 1) % NT, 0:65], j, 64))
                for mi, (lhsT, rhs, j, pb) in enumerate(mms):
                    out = (cp[:, j, 0:65] if pb is None
                           else cp[pb:pb + 64, j, 0:65])
                    nc.tensor.matmul(out, lhsT, rhs,
                                     start=(mi == 0), stop=(mi == len(mms) - 1),
                                     skip_group_check=True)
                if st_.pop("last_tail"):
                    nc.scalar.copy(st_["ctx"][:, 4 * s:4 * s + 4, :],
                                   cp[:, :, 0:65])
                else:
                    nc.vector.tensor_copy(st_["ctx"][:, 4 * s:4 * s + 4, :],
                                          cp[:, :, 0:65])
                st_["nseg"] += 1
                if i == PER_CORE - 1:
                    # tail pair fills slots in order 0..5,6,7: ship the bulk
                    # early, the last 8 tile-slots via a separate tensor
                    if st_["nseg"] == 6:
                        nc.sync.dma_start(out_d[i][:, 0:24], st_["ctx"][:, 0:24])
                    elif st_["nseg"] == 8:
                        nc.sync.dma_start(out_t[:], st_["ctx"][:, 24:NT])
                elif st_["nseg"] == 8:
                    nc.sync.dma_start(out_d[i], st_["ctx"][:])

            states = {}
            last = PER_CORE - 1
            ident_loaded = [False]
            # pair 0: strict phase order - the Act queue is in-order, so a
            # global act waiting on the global gather must not sit in front
            # of local acts whose data is already there
            states[0] = loads(0)
            nc.sync.dma_start(ident[:], ident_d[:])
            states[1] = loads(1)
            vt_init(states[0])
            exp_tiles(states[0])
            for p in range(4):
                gtok_pack(states[0], p)
            for p in range(8):
                local_pack(states[0], p, ms_dve=True)
                if p >= 4:
                    vt_pack(states[0], p - 4)   # local vt: kTL ready
            for p in range(8):
                global_pack(states[0], p,
                             engs={0: "v", 2: "v", 4: "v", 6: "v"})
                if p >= 4:
                    vt_pack(states[0], p)       # global vt: kTG ready
            vt_init(states[1])
            # middle pairs: interleaved with previous pair's PV
            for i in range(1, PER_CORE):
                if i == 1:
                    states[2] = loads(2)
                if i != 1:
                    vt_init(states[i])
                exp_tiles(states[i])
                # last pair: emit pack 7 first so the wrap PV segments can
                # start before the section ends
                rot = (7, 0, 1, 2, 3, 4, 5, 6) if i == last else tuple(range(8))
                for sl in range(8):
                    p = rot[sl]
                    if sl < 4:
                        gtok_pack(states[i], sl)
                    local_pack(states[i], p)
                    global_pack(states[i], p,
                                engs=GPACK_ENG if i != last else GPACK_ENG_LAST)
                    if sl < 2:
                        vt_pack(states[i], 2 * sl)
                        vt_pack(states[i], 2 * sl + 1)
                        if i == last:
                            vt_pack(states[i], 2 * sl + 4)
                            vt_pack(states[i], 2 * sl + 5)
                    elif i != last and sl >= 4:
                        vt_pack(states[i], sl)
                    pv_group(states[i - 1], i - 1,
                             (1, 2, 3, 4, 5, 6, 7, 0)[sl])
                    if i == last and sl >= 2:
                        pv_group(states[i], i, sl - 2)
            for s in (6, 7):
                pv_group(states[last], last, s)

    nc.compile()
    return nc


_CACHED = None


def _get_program():
    global _CACHED
    if _CACHED is None:
        _CACHED = build_program()
    return _CACHED


def _prep_core_inputs(q, k, v, gk, gv, lidx, gidx, pairs):
    """Build one core's input dict for its list of (n,h) pairs."""
    bf = ml_dtypes.bfloat16
    qTh = np.empty((PER_CORE, 64, QH_W), dtype=bf)
    kv = np.empty((PER_CORE, T, 128), dtype=bf)
    gkT = np.empty((PER_CORE, 64, 64), dtype=bf)
    gv1 = np.zeros((PER_CORE, 128, 2, 65), dtype=bf)
    li = np.empty((PER_CORE, 128, 256), dtype=np.int16)
    gi = np.empty((PER_CORE, 128, 256), dtype=np.int16)
    for s, (n, h) in enumerate(pairs):
        qt = np.ascontiguousarray(q[n, h].T)            # (64, T) f32
        qth = np.concatenate([qt[:, T - 128:], qt, qt[:, :256]], axis=1)
        qTh[s] = qth.astype(bf)
        kv[s, :, 0:64] = k[n, h].astype(bf)
        kv[s, :, 64:128] = v[n, h].astype(bf)
        gkT[s] = np.ascontiguousarray(gk[n, h].T).astype(bf)
        g1 = np.concatenate([gv[n, h], np.ones((64, 1), np.float32)],
                            axis=1).astype(bf)
        gv1[s, 0:64, 0] = g1      # parity 0: top half live
        gv1[s, 64:128, 1] = g1    # parity 1: bottom half live
        for arr, src in ((li, lidx), (gi, gidx)):
            ix = src[n, h, :, 0].astype(np.int16)       # (T,)
            arr[s] = np.tile(ix.reshape(T // 16, 16).T, (8, 1))
    ident = np.eye(128, dtype=bf)
    return {"qTh": qTh, "kvtab": kv, "gkT": gkT, "gv1": gv1,
            "lidx": li, "gidx": gi, "ident": ident}


def kernel(query_layer, key_layer, value_layer, attention_mask, local_idx,
           global_idx, global_key, global_value, global_mask):
    # attention_mask / global_mask are all-zero in this problem's input spec;
    # they contribute nothing to the scores and are not shipped to the device.
    q = np.asarray(query_layer, np.float32)
    k = np.asarray(key_layer, np.float32)
    v = np.asarray(value_layer, np.float32)
    gk = np.asarray(global_key, np.float32)
    gv = np.asarray(global_value, np.float32)
    li = np.asarray(local_idx)
    gi = np.asarray(global_idx)

    nc = _get_program()
    in_maps = []
    for m in range(NCORES):
        pairs = [((3 * m + s) // H, (3 * m + s) % H) for s in range(PER_CORE)]
        in_maps.append(_prep_core_inputs(q, k, v, gk, gv, li, gi, pairs))
    res = bass_utils.run_bass_kernel_spmd(nc, in_maps, core_ids=list(range(NCORES)))

    out = np.empty((N, H, T, D), np.float32)
    for m in range(NCORES):
        ctxT = np.asarray(res.results[m]["ctxT"]).astype(np.float32)
        tail = np.asarray(res.results[m]["ctxTail"]).astype(np.float32)
        ctxT[PER_CORE - 1, :, 24:] = tail
        for s in range(PER_CORE):
            n, h = (3 * m + s) // H, (3 * m + s) % H
            a = ctxT[s].transpose(1, 0, 2).reshape(T, 65)  # q-major rows
            out[n, h] = a[:, :64] / a[:, 64:65]
    return out


# revision 11
# speedup vs baseline: 1.7083x; 1.0006x over previous
"""BlockGlobalAttentionProduct Trainium2 kernel (v2).

Sharding: 24 (n,h) pairs across 8 cores, 3 per core. Per (n,h):
  - kv table rows in DRAM: [K bf16 64 | V bf16 64], 256B/row.
  - transpose-mode dma_gather lands K^T (d on partitions 0:64) and V^T
    (partitions 64:128) directly in SBUF - no PE K-transposes, no PSUM->SBUF
    K copies. A [64,128] PE transpose per key tile turns V^T back into V
    rows ([V|1] with a memset ones column -> denominator in row 64).
  - QK in bf16 -> PSUM f32 -> exp to bf16 score tiles: exact ScalarE exp
    for pair 0 (filling the gather-bound startup) and for 2 of 8 global
    packs on later pairs; the rest via a DVE bit-trick (trunc(x*S+B) as
    int16 is the bf16 bit pattern of ~e^x, max rel err ~3.5%).
    Local-window staircase corners zeroed by Pool memsets.
  - PV in ctx orientation (probs^T stationary, [V|1] moving): 7 matmuls of
    65 output columns per 128-query block, accumulated in a [128, 4, 128]
    PSUM tile - about half the output columns of the scores^T orientation,
    no segment-boundary splits, and the result lands q-major.
  - host divides by the column-64 denominator during unshard (no transpose).
"""

import sys

sys.path.insert(0, "/opt/trn_rl_repo")

import numpy as np
import ml_dtypes

import concourse.bacc as bacc
import concourse.mybir as mybir
from concourse import bass, tile, bass_utils, library_config

N, H, T, D = 2, 12, 4096, 64
NH = N * H
NCORES = 8
PER_CORE = NH // NCORES   # 3
NT = T // 128             # 32 key tiles per table
QH_W = 128 + T + 256      # q^T halo: cols [-128, 4352)

BF16 = mybir.dt.bfloat16
F32 = mybir.dt.float32
I16 = mybir.dt.int16
EXP = mybir.ActivationFunctionType.Exp
# bf16-Schraudolph: trunc(x*S16 + B16) as int16 is the bf16 bit pattern of
# ~e^(x/8) (max rel err ~3.5%); used on the DVE for part of the exp work
S16 = float(16.0 * np.log2(np.e))
B16 = float(128.0 * (127.0 - 0.0430))
# per-pack exp engine for the global table on pipelined pairs:
# v=DVE bit-trick, p=Pool bit-trick, a=ScalarE exact
GPACK_ENG = {0: "v", 1: "v", 2: "v", 3: "a", 4: "v", 5: "v", 6: "v", 7: "a"}
# last pair: DVE takes the early packs, the final packs run exact on the
# by-then-idle ScalarE so the tail PV isn't gated on a busy DVE
GPACK_ENG_LAST = {0: "v", 1: "v", 2: "v", 3: "v", 4: "v", 5: "v"}


def build_program():
    nc = bacc.Bacc("TRN2", target_bir_lowering=False, debug=False,
                   num_devices=NCORES)

    qTh = nc.dram_tensor("qTh", [PER_CORE, 64, QH_W], BF16, kind="ExternalInput")
    kvtab = nc.dram_tensor("kvtab", [PER_CORE, T, 128], BF16, kind="ExternalInput")
    gkT_d = nc.dram_tensor("gkT", [PER_CORE, 64, 64], BF16, kind="ExternalInput")
    gv1_d = nc.dram_tensor("gv1", [PER_CORE, 128, 2, 65], BF16, kind="ExternalInput")
    lidx_d = nc.dram_tensor("lidx", [PER_CORE, 128, 256], I16, kind="ExternalInput")
    gidx_d = nc.dram_tensor("gidx", [PER_CORE, 128, 256], I16, kind="ExternalInput")
    ident_d = nc.dram_tensor("ident", [128, 128], BF16, kind="ExternalInput")
    out_d = nc.dram_tensor("ctxT", [PER_CORE, 128, NT, 65], BF16, kind="ExternalOutput")
    out_t = nc.dram_tensor("ctxTail", [128, 8, 65], BF16, kind="ExternalOutput")

    with tile.TileContext(nc) as tc:
        with (
            tc.tile_pool(name="const", bufs=1) as constp,
            tc.tile_pool(name="land", bufs=2) as land,
            tc.tile_pool(name="ktp", bufs=2) as ktp,
            tc.tile_pool(name="v1p", bufs=3) as v1p,
            tc.tile_pool(name="expp", bufs=2) as expp,
            tc.tile_pool(name="outp", bufs=2) as outp,
            tc.tile_pool(name="psq", bufs=3, space="PSUM") as psq,
            tc.tile_pool(name="psa", bufs=2, space="PSUM") as psa,
        ):
            ident = constp.tile([128, 128], BF16, tag="ident")
            lib_i = nc.gpsimd.load_library(library_config.mlp)
            first_gather = [True]

            def loads(i):
                q_sb = land.tile([64, QH_W], BF16, tag="q")
                gkT = land.tile([64, 64], BF16, tag="gkT")
                gv1 = land.tile([128, 2, 65], BF16, tag="gv1")
                li = land.tile([128, 256], I16, tag="li")
                gi = land.tile([128, 256], I16, tag="gi")
                kTL = ktp.tile([128, T], BF16, tag="kTL")
                kTG = ktp.tile([128, T], BF16, tag="kTG")
                dma_q = nc.gpsimd if i == 0 else nc.sync
                dma_q.dma_start(li[:], lidx_d[i])
                dma_q.dma_start(gi[:], gidx_d[i])
                nc.sync.dma_start(gkT[:], gkT_d[i])
                nc.sync.dma_start(q_sb[:, 0:2304], qTh[i][:, 0:2304])
                nc.sync.dma_start(q_sb[:, 2304:QH_W], qTh[i][:, 2304:QH_W])
                nc.sync.dma_start(gv1[:], gv1_d[i])
                for kT, idx in ((kTL, li), (kTG, gi)):
                    # pair 0 local table in quarters (earliest compute
                    # start); everything else in halves (less desc-gen)
                    nh = 4 if (i == 0 and kT is kTL) else 2
                    w = T // nh
                    for h in range(nh):
                        dst = kT[:, w * h:w * (h + 1)].rearrange(
                            "p (a b) -> p a b", a=1)
                        g = nc.gpsimd.dma_gather(
                            dst, kvtab[i],
                            idx[:, (w // 16) * h:(w // 16) * (h + 1)],
                            w, w, 128, transpose=True,
                            single_packet=False)
                        if first_gather[0]:
                            from concourse.tile_rust import add_dep_helper
                            add_dep_helper(lib_i.ins, g.ins,
                                           reason="lib before gather")
                            first_gather[0] = False
                return dict(q=q_sb, gkT=gkT, gv1=gv1, kTL=kTL, kTG=kTG)

            def vt_init(st_):
                v1L = v1p.tile([128, NT, 66], BF16, tag="v1L")
                v1G = v1p.tile([128, NT, 66], BF16, tag="v1G")
                nc.gpsimd.memset(v1L[:, :, 64:65], 1.0)
                nc.gpsimd.memset(v1G[:, :, 64:65], 1.0)
                st_["v1L"], st_["v1G"] = v1L, v1G

            def vt_pack(st_, j):
                # V rows: transpose V^T (partitions 64:128 of the gathered
                # tiles) back to keys-on-partitions, 8 key tiles per psum pack
                kT = st_["kTL"] if j < 4 else st_["kTG"]
                v1 = st_["v1L"] if j < 4 else st_["v1G"]
                p = j % 4
                tp = psq.tile([128, 512], BF16, tag="qk")
                for k in range(8):
                    c = 8 * p + k
                    nc.tensor.transpose(
                        tp[:, 64 * k:64 * k + 64],
                        in_=kT[64:128, 128 * c:128 * c + 128],
                        identity=ident[64:128, 64:128])
                nc.vector.tensor_copy(
                    v1[:, 8 * p:8 * p + 8, 0:64],
                    tp[:].rearrange("p (a b) -> p a b", b=64))

            def exp_tiles(st_):
                eL = expp.tile([128, NT, 256], BF16, tag="expL")
                eG = expp.tile([128, NT, 384], BF16, tag="expG")
                eT = expp.tile([128, 4, 512], BF16, tag="expT")
                st_["expL"], st_["expG"], st_["expT"] = eL, eG, eT

            def local_pack(st_, p, ms_dve=False):
                st = psq.tile([128, 1024], F32, tag="qk")
                for j in range(4):
                    c = 4 * p + j
                    nc.tensor.matmul(
                        st[:, 256 * j:256 * j + 256],
                        st_["kTL"][0:64, 128 * c:128 * c + 128],
                        st_["q"][:, 64 + 128 * c:64 + 128 * c + 256],
                        start=True, stop=True)
                nc.scalar.activation(
                    st_["expL"][:, 4 * p:4 * p + 4, :],
                    st[:, 0:1024].rearrange("p (a b) -> p a b", b=256),
                    EXP, scale=0.125)
                ms_eng = nc.vector if ms_dve else nc.gpsimd
                ms_eng.memset(st_["expL"][64:128, 4 * p:4 * p + 4, 0:64], 0)
                ms_eng.memset(st_["expL"][0:64, 4 * p:4 * p + 4, 192:256], 0)

            def global_pack(st_, p, engs=None):
                # two half-packs of 2 tiles each: bank-aligned 384-col chunks
                # (no split matmuls) and a finer psum-pool rotation
                eng = (engs or {}).get(p, "a")
                for half in range(2):
                    st = psq.tile([128, 1024], F32, tag="qk")
                    for j in range(2):
                        t = 4 * p + 2 * half + j
                        nc.tensor.matmul(
                            st[:, 512 * j:512 * j + 384],
                            st_["kTG"][0:64, 128 * t:128 * t + 128],
                            st_["q"][:, 128 * t:128 * t + 384],
                            start=True, stop=True)
                    src_v = st[:].rearrange("p (a b) -> p a b", b=512)[:, :, 0:384]
                    o = st_["expG"][:, 4 * p + 2 * half:4 * p + 2 * half + 2, :]
                    if eng == "a":
                        nc.scalar.activation(o, src_v, EXP, scale=0.125)
                    else:
                        e_ = nc.vector if eng == "v" else nc.gpsimd
                        e_.tensor_scalar(o.bitcast(I16), src_v, S16, B16,
                                         mybir.AluOpType.mult,
                                         mybir.AluOpType.add)

            def gtok_pack(st_, p):
                st = psq.tile([128, 512], F32, tag="qk")
                for j in range(2):
                    g = 2 * p + j
                    nc.tensor.matmul(
                        st[64 * j:64 * j + 64, 0:512],
                        st_["gkT"][:],
                        st_["q"][:, 128 + 512 * g:128 + 512 * g + 512],
                        start=True, stop=True, tile_position=(0, 64 * j))
                nc.scalar.activation(st_["expT"][:, p, :], st[:],
                                     EXP, scale=0.125)

            def pv_group(st_, i, s):
                """ctx rows for query blocks 4s..4s+4 (probs^T stationary,
                V1 moving: out columns = 65 per piece instead of the window
                width, and the output lands q-major)."""
                if "ctx" not in st_:
                    ctx = outp.tile([128, NT, 65], BF16, tag="ctx")
                    st_["ctx"] = ctx
                    st_["nseg"] = 0
                cp = psa.tile([128, 4, 128], F32, tag="cp")  # bank-aligned
                st_["last_tail"] = (i == PER_CORE - 1 and s >= 6)
                mms = []
                for j in range(4):
                    b = 4 * s + j
                    # full-128q pieces first: each block's bytes are zeroed
                    # by its first (full-partition) accumulating matmul
                    mms.append((st_["expL"][:, b % NT, 64:192],
                                st_["v1L"][:, b % NT, 0:65], j, None))
                    mms.append((st_["expT"][:, s // 2, 128 * j:128 * j + 128],
                                st_["gv1"][:, s % 2, :], j, None))
                    for t, c0 in ((b - 1, 256), (b, 128), (b + 1, 0)):
                        mms.append((st_["expG"][:, t % NT, c0:c0 + 128],
                                    st_["v1G"][:, t % NT, 0:65], j, None))
                    mms.append((st_["expL"][:, (b - 1) % NT, 192:256],
                                st_["v1L"][:, (b - 1) % NT, 0:65], j, 0))
                    mms.append((st_["expL"][:, (b + 1) % NT, 0:64],
                                st_["v1L"][:, (b + 1) % NT, 0:65], j, 64))
                for mi, (lhsT, rhs, j, pb) in enumerate(mms):
                    out = (cp[:, j, 0:65] if pb is None
                           else cp[pb:pb + 64, j, 0:65])
                    nc.tensor.matmul(out, lhsT, rhs,
                                     start=(mi == 0), stop=(mi == len(mms) - 1),
                                     skip_group_check=True)
                if st_.pop("last_tail"):
                    nc.scalar.copy(st_["ctx"][:, 4 * s:4 * s + 4, :],
                                   cp[:, :, 0:65])
                else:
                    nc.vector.tensor_copy(st_["ctx"][:, 4 * s:4 * s + 4, :],
                                          cp[:, :, 0:65])
                st_["nseg"] += 1
                if i == PER_CORE - 1:
                    # tail pair fills slots in order 0..5,6,7: ship the bulk
                    # early, the last 8 tile-slots via a separate tensor
                    if st_["nseg"] == 6:
                        nc.sync.dma_start(out_d[i][:, 0:24], st_["ctx"][:, 0:24])
                    elif st_["nseg"] == 8:
                        nc.sync.dma_start(out_t[:], st_["ctx"][:, 24:NT])
                elif st_["nseg"] == 8:
                    nc.sync.dma_start(out_d[i], st_["ctx"][:])

            states = {}
            last = PER_CORE - 1
            ident_loaded = [False]
            # pair 0: strict phase order - the Act queue is in-order, so a
            # global act waiting on the global gather must not sit in front
            # of local acts whose data is already there
            states[0] = loads(0)
            nc.sync.dma_start(ident[:], ident_d[:])
            states[1] = loads(1)
            vt_init(states[0])
            exp_tiles(states[0])
            for p in range(4):
                gtok_pack(states[0], p)
            for p in range(8):
                local_pack(states[0], p, ms_dve=True)
                if p >= 4:
                    vt_pack(states[0], p - 4)   # local vt: kTL ready
            for p in range(8):
                global_pack(states[0], p,
                             engs={0: "v", 2: "v", 4: "v", 6: "v"})
                if p >= 4:
                    vt_pack(states[0], p)       # global vt: kTG ready
            vt_init(states[1])
            # middle pairs: interleaved with previous pair's PV
            for i in range(1, PER_CORE):
                if i == 1:
                    states[2] = loads(2)
                if i != 1:
                    vt_init(states[i])
                exp_tiles(states[i])
                # last pair: emit pack 7 first so the wrap PV segments can
                # start before the section ends
                rot = (7, 0, 1, 2, 3, 4, 5, 6) if i == last else tuple(range(8))
                for sl in range(8):
                    p = rot[sl]
                    if sl < 4:
                        gtok_pack(states[i], sl)
                    local_pack(states[i], p)
                    global_pack(states[i], p,
                                engs=GPACK_ENG if i != last else GPACK_ENG_LAST)
                    if sl < 2:
                        vt_pack(states[i], 2 * sl)
                        vt_pack(states[i], 2 * sl + 1)
                        if i == last:
                            vt_pack(states[i], 2 * sl + 4)
                            vt_pack(states[i], 2 * sl + 5)
                    elif i != last and sl >= 4:
                        vt_pack(states[i], sl)
                    pv_group(states[i - 1], i - 1,
                             (0, 1, 2, 3, 4, 5, 6, 7)[sl])
                    if i == last and sl >= 2:
                        pv_group(states[i], i, sl - 2)
            for s in (6, 7):
                pv_group(states[last], last, s)

    nc.compile()
    return nc


_CACHED = None


def _get_program():
    global _CACHED
    if _CACHED is None:
        _CACHED = build_program()
    return _CACHED


def _prep_core_inputs(q, k, v, gk, gv, lidx, gidx, pairs):
    """Build one core's input dict for its list of (n,h) pairs."""
    bf = ml_dtypes.bfloat16
    qTh = np.empty((PER_CORE, 64, QH_W), dtype=bf)
    kv = np.empty((PER_CORE, T, 128), dtype=bf)
    gkT = np.empty((PER_CORE, 64, 64), dtype=bf)
    gv1 = np.zeros((PER_CORE, 128, 2, 65), dtype=bf)
    li = np.empty((PER_CORE, 128, 256), dtype=np.int16)
    gi = np.empty((PER_CORE, 128, 256), dtype=np.int16)
    for s, (n, h) in enumerate(pairs):
        qt = np.ascontiguousarray(q[n, h].T)            # (64, T) f32
        qth = np.concatenate([qt[:, T - 128:], qt, qt[:, :256]], axis=1)
        qTh[s] = qth.astype(bf)
        kv[s, :, 0:64] = k[n, h].astype(bf)
        kv[s, :, 64:128] = v[n, h].astype(bf)
        gkT[s] = np.ascontiguousarray(gk[n, h].T).astype(bf)
        g1 = np.concatenate([gv[n, h], np.ones((64, 1), np.float32)],
                            axis=1).astype(bf)
        gv1[s, 0:64, 0] = g1      # parity 0: top half live
        gv1[s, 64:128, 1] = g1    # parity 1: bottom half live
        for arr, src in ((li, lidx), (gi, gidx)):
            ix = src[n, h, :, 0].astype(np.int16)       # (T,)
            arr[s] = np.tile(ix.reshape(T // 16, 16).T, (8, 1))
    ident = np.eye(128, dtype=bf)
    return {"qTh": qTh, "kvtab": kv, "gkT": gkT, "gv1": gv1,
            "lidx": li, "gidx": gi, "ident": ident}


def kernel(query_layer, key_layer, value_layer, attention_mask, local_idx,
           global_idx, global_key, global_value, global_mask):
    # attention_mask / global_mask are all-zero in this problem's input spec;
    # they contribute nothing to the scores and are not shipped to the device.
    q = np.asarray(query_layer, np.float32)
    k = np.asarray(key_layer, np.float32)
    v = np.asarray(value_layer, np.float32)
    gk = np.asarray(global_key, np.float32)
    gv = np.asarray(global_value, np.float32)
    li = np.asarray(local_idx)
    gi = np.asarray(global_idx)

    nc = _get_program()
    in_maps = []
    for m in range(NCORES):
        pairs = [((3 * m + s) // H, (3 * m + s) % H) for s in range(PER_CORE)]
        in_maps.append(_prep_core_inputs(q, k, v, gk, gv, li, gi, pairs))
    res = bass_utils.run_bass_kernel_spmd(nc, in_maps, core_ids=list(range(NCORES)))

    out = np.empty((N, H, T, D), np.float32)
    for m in range(NCORES):
        ctxT = np.asarray(res.results[m]["ctxT"]).astype(np.float32)
        tail = np.asarray(res.results[m]["ctxTail"]).astype(np.float32)
        ctxT[PER_CORE - 1, :, 24:] = tail
        for s in range(PER_CORE):
            n, h = (3 * m + s) // H, (3 * m + s) % H
            a = ctxT[s].transpose(1, 0, 2).reshape(T, 65)  # q-major rows
            out[n, h] = a[:, :64] / a[:, 64:65]
    return out


# revision 12
# speedup vs baseline: 1.7105x; 1.0012x over previous
"""BlockGlobalAttentionProduct Trainium2 kernel (v2).

Sharding: 24 (n,h) pairs across 8 cores, 3 per core. Per (n,h):
  - kv table rows in DRAM: [K bf16 64 | V bf16 64], 256B/row.
  - transpose-mode dma_gather lands K^T (d on partitions 0:64) and V^T
    (partitions 64:128) directly in SBUF - no PE K-transposes, no PSUM->SBUF
    K copies. A [64,128] PE transpose per key tile turns V^T back into V
    rows ([V|1] with a memset ones column -> denominator in row 64).
  - QK in bf16 -> PSUM f32 -> exp to bf16 score tiles: exact ScalarE exp
    for pair 0 (filling the gather-bound startup) and for 2 of 8 global
    packs on later pairs; the rest via a DVE bit-trick (trunc(x*S+B) as
    int16 is the bf16 bit pattern of ~e^x, max rel err ~3.5%).
    Local-window staircase corners zeroed by Pool memsets.
  - PV in ctx orientation (probs^T stationary, [V|1] moving): 7 matmuls of
    65 output columns per 128-query block, accumulated in a [128, 4, 128]
    PSUM tile - about half the output columns of the scores^T orientation,
    no segment-boundary splits, and the result lands q-major.
  - host divides by the column-64 denominator during unshard (no transpose).
"""

import sys

sys.path.insert(0, "/opt/trn_rl_repo")

import numpy as np
import ml_dtypes

import concourse.bacc as bacc
import concourse.mybir as mybir
from concourse import bass, tile, bass_utils, library_config

N, H, T, D = 2, 12, 4096, 64
NH = N * H
NCORES = 8
PER_CORE = NH // NCORES   # 3
NT = T // 128             # 32 key tiles per table
QH_W = 128 + T + 256      # q^T halo: cols [-128, 4352)

BF16 = mybir.dt.bfloat16
F32 = mybir.dt.float32
I16 = mybir.dt.int16
EXP = mybir.ActivationFunctionType.Exp
# bf16-Schraudolph: trunc(x*S16 + B16) as int16 is the bf16 bit pattern of
# ~e^(x/8) (max rel err ~3.5%); used on the DVE for part of the exp work
S16 = float(16.0 * np.log2(np.e))
B16 = float(128.0 * (127.0 - 0.0430))
# per-pack exp engine for the global table on pipelined pairs:
# v=DVE bit-trick, p=Pool bit-trick, a=ScalarE exact
GPACK_ENG = {0: "v", 1: "v", 2: "v", 3: "a", 4: "v", 5: "v", 6: "v", 7: "a"}
# last pair: DVE takes the early packs, the final packs run exact on the
# by-then-idle ScalarE so the tail PV isn't gated on a busy DVE
GPACK_ENG_LAST = {0: "v", 1: "v", 2: "v", 3: "v", 4: "v", 5: "v"}


def build_program():
    nc = bacc.Bacc("TRN2", target_bir_lowering=False, debug=False,
                   num_devices=NCORES)

    qTh = nc.dram_tensor("qTh", [PER_CORE, 64, QH_W], BF16, kind="ExternalInput")
    kvtab = nc.dram_tensor("kvtab", [PER_CORE, T, 128], BF16, kind="ExternalInput")
    gkT_d = nc.dram_tensor("gkT", [PER_CORE, 64, 64], BF16, kind="ExternalInput")
    gv1_d = nc.dram_tensor("gv1", [PER_CORE, 128, 2, 65], BF16, kind="ExternalInput")
    lidx_d = nc.dram_tensor("lidx", [PER_CORE, 128, 256], I16, kind="ExternalInput")
    gidx_d = nc.dram_tensor("gidx", [PER_CORE, 128, 256], I16, kind="ExternalInput")
    ident_d = nc.dram_tensor("ident", [128, 128], BF16, kind="ExternalInput")
    out_d = nc.dram_tensor("ctxT", [PER_CORE, 128, NT, 65], BF16, kind="ExternalOutput")
    out_t = nc.dram_tensor("ctxTail", [128, 8, 65], BF16, kind="ExternalOutput")

    with tile.TileContext(nc) as tc:
        with (
            tc.tile_pool(name="const", bufs=1) as constp,
            tc.tile_pool(name="land", bufs=2) as land,
            tc.tile_pool(name="ktp", bufs=2) as ktp,
            tc.tile_pool(name="v1p", bufs=2) as v1p,
            tc.tile_pool(name="expp", bufs=2) as expp,
            tc.tile_pool(name="outp", bufs=2) as outp,
            tc.tile_pool(name="psq", bufs=3, space="PSUM") as psq,
            tc.tile_pool(name="psa", bufs=2, space="PSUM") as psa,
        ):
            ident = constp.tile([128, 128], BF16, tag="ident")
            lib_i = nc.gpsimd.load_library(library_config.mlp)
            first_gather = [True]

            def loads(i):
                q_sb = land.tile([64, QH_W], BF16, tag="q")
                gkT = land.tile([64, 64], BF16, tag="gkT")
                gv1 = land.tile([128, 2, 65], BF16, tag="gv1")
                li = land.tile([128, 256], I16, tag="li")
                gi = land.tile([128, 256], I16, tag="gi")
                kTL = ktp.tile([128, T], BF16, tag="kTL")
                kTG = ktp.tile([128, T], BF16, tag="kTG")
                dma_q = nc.gpsimd if i == 0 else nc.sync
                dma_q.dma_start(li[:], lidx_d[i])
                dma_q.dma_start(gi[:], gidx_d[i])
                nc.sync.dma_start(gkT[:], gkT_d[i])
                nc.sync.dma_start(q_sb[:, 0:2304], qTh[i][:, 0:2304])
                nc.sync.dma_start(q_sb[:, 2304:QH_W], qTh[i][:, 2304:QH_W])
                nc.sync.dma_start(gv1[:], gv1_d[i])
                for kT, idx in ((kTL, li), (kTG, gi)):
                    # pair 0 local table in quarters (earliest compute
                    # start); everything else in halves (less desc-gen)
                    nh = 4 if (i == 0 and kT is kTL) else 2
                    w = T // nh
                    for h in range(nh):
                        dst = kT[:, w * h:w * (h + 1)].rearrange(
                            "p (a b) -> p a b", a=1)
                        g = nc.gpsimd.dma_gather(
                            dst, kvtab[i],
                            idx[:, (w // 16) * h:(w // 16) * (h + 1)],
                            w, w, 128, transpose=True,
                            single_packet=False)
                        if first_gather[0]:
                            from concourse.tile_rust import add_dep_helper
                            add_dep_helper(lib_i.ins, g.ins,
                                           reason="lib before gather")
                            first_gather[0] = False
                return dict(q=q_sb, gkT=gkT, gv1=gv1, kTL=kTL, kTG=kTG)

            def vt_init(st_):
                v1L = v1p.tile([128, NT, 66], BF16, tag="v1L")
                v1G = v1p.tile([128, NT, 66], BF16, tag="v1G")
                nc.gpsimd.memset(v1L[:, :, 64:65], 1.0)
                nc.gpsimd.memset(v1G[:, :, 64:65], 1.0)
                st_["v1L"], st_["v1G"] = v1L, v1G

            def vt_pack(st_, j):
                # V rows: transpose V^T (partitions 64:128 of the gathered
                # tiles) back to keys-on-partitions, 8 key tiles per psum pack
                kT = st_["kTL"] if j < 4 else st_["kTG"]
                v1 = st_["v1L"] if j < 4 else st_["v1G"]
                p = j % 4
                tp = psq.tile([128, 512], BF16, tag="qk")
                for k in range(8):
                    c = 8 * p + k
                    nc.tensor.transpose(
                        tp[:, 64 * k:64 * k + 64],
                        in_=kT[64:128, 128 * c:128 * c + 128],
                        identity=ident[64:128, 64:128])
                nc.vector.tensor_copy(
                    v1[:, 8 * p:8 * p + 8, 0:64],
                    tp[:].rearrange("p (a b) -> p a b", b=64))

            def exp_tiles(st_):
                eL = expp.tile([128, NT, 256], BF16, tag="expL")
                eG = expp.tile([128, NT, 384], BF16, tag="expG")
                eT = expp.tile([128, 4, 512], BF16, tag="expT")
                st_["expL"], st_["expG"], st_["expT"] = eL, eG, eT

            def local_pack(st_, p, ms_dve=False):
                st = psq.tile([128, 1024], F32, tag="qk")
                for j in range(4):
                    c = 4 * p + j
                    nc.tensor.matmul(
                        st[:, 256 * j:256 * j + 256],
                        st_["kTL"][0:64, 128 * c:128 * c + 128],
                        st_["q"][:, 64 + 128 * c:64 + 128 * c + 256],
                        start=True, stop=True)
                nc.scalar.activation(
                    st_["expL"][:, 4 * p:4 * p + 4, :],
                    st[:, 0:1024].rearrange("p (a b) -> p a b", b=256),
                    EXP, scale=0.125)
                ms_eng = nc.vector if ms_dve else nc.gpsimd
                ms_eng.memset(st_["expL"][64:128, 4 * p:4 * p + 4, 0:64], 0)
                ms_eng.memset(st_["expL"][0:64, 4 * p:4 * p + 4, 192:256], 0)

            def global_pack(st_, p, engs=None):
                # two half-packs of 2 tiles each: bank-aligned 384-col chunks
                # (no split matmuls) and a finer psum-pool rotation
                eng = (engs or {}).get(p, "a")
                for half in range(2):
                    st = psq.tile([128, 1024], F32, tag="qk")
                    for j in range(2):
                        t = 4 * p + 2 * half + j
                        nc.tensor.matmul(
                            st[:, 512 * j:512 * j + 384],
                            st_["kTG"][0:64, 128 * t:128 * t + 128],
                            st_["q"][:, 128 * t:128 * t + 384],
                            start=True, stop=True)
                    src_v = st[:].rearrange("p (a b) -> p a b", b=512)[:, :, 0:384]
                    o = st_["expG"][:, 4 * p + 2 * half:4 * p + 2 * half + 2, :]
                    if eng == "a":
                        nc.scalar.activation(o, src_v, EXP, scale=0.125)
                    else:
                        e_ = nc.vector if eng == "v" else nc.gpsimd
                        e_.tensor_scalar(o.bitcast(I16), src_v, S16, B16,
                                         mybir.AluOpType.mult,
                                         mybir.AluOpType.add)

            def gtok_pack(st_, p):
                st = psq.tile([128, 512], F32, tag="qk")
                for j in range(2):
                    g = 2 * p + j
                    nc.tensor.matmul(
                        st[64 * j:64 * j + 64, 0:512],
                        st_["gkT"][:],
                        st_["q"][:, 128 + 512 * g:128 + 512 * g + 512],
                        start=True, stop=True, tile_position=(0, 64 * j))
                nc.scalar.activation(st_["expT"][:, p, :], st[:],
                                     EXP, scale=0.125)

            def pv_group(st_, i, s):
                """ctx rows for query blocks 4s..4s+4 (probs^T stationary,
                V1 moving: out columns = 65 per piece instead of the window
                width, and the output lands q-major)."""
                if "ctx" not in st_:
                    ctx = outp.tile([128, NT, 65], BF16, tag="ctx")
                    st_["ctx"] = ctx
                    st_["nseg"] = 0
                cp = psa.tile([128, 4, 128], F32, tag="cp")  # bank-aligned
                st_["last_tail"] = (i == PER_CORE - 1 and s >= 6)
                mms = []
                for j in range(4):
                    b = 4 * s + j
                    # full-128q pieces first: each block's bytes are zeroed
                    # by its first (full-partition) accumulating matmul
                    mms.append((st_["expL"][:, b % NT, 64:192],
                                st_["v1L"][:, b % NT, 0:65], j, None))
                    mms.append((st_["expT"][:, s // 2, 128 * j:128 * j + 128],
                                st_["gv1"][:, s % 2, :], j, None))
                    for t, c0 in ((b - 1, 256), (b, 128), (b + 1, 0)):
                        mms.append((st_["expG"][:, t % NT, c0:c0 + 128],
                                    st_["v1G"][:, t % NT, 0:65], j, None))
                    mms.append((st_["expL"][:, (b - 1) % NT, 192:256],
                                st_["v1L"][:, (b - 1) % NT, 0:65], j, 0))
                    mms.append((st_["expL"][:, (b + 1) % NT, 0:64],
                                st_["v1L"][:, (b + 1) % NT, 0:65], j, 64))
                for mi, (lhsT, rhs, j, pb) in enumerate(mms):
                    out = (cp[:, j, 0:65] if pb is None
                           else cp[pb:pb + 64, j, 0:65])
                    nc.tensor.matmul(out, lhsT, rhs,
                                     start=(mi == 0), stop=(mi == len(mms) - 1),
                                     skip_group_check=True)
                if st_.pop("last_tail"):
                    nc.scalar.copy(st_["ctx"][:, 4 * s:4 * s + 4, :],
                                   cp[:, :, 0:65])
                else:
                    nc.vector.tensor_copy(st_["ctx"][:, 4 * s:4 * s + 4, :],
                                          cp[:, :, 0:65])
                st_["nseg"] += 1
                if i == PER_CORE - 1:
                    # tail pair fills slots in order 0..5,6,7: ship the bulk
                    # early, the last 8 tile-slots via a separate tensor
                    if st_["nseg"] == 6:
                        nc.sync.dma_start(out_d[i][:, 0:24], st_["ctx"][:, 0:24])
                    elif st_["nseg"] == 8:
                        nc.sync.dma_start(out_t[:], st_["ctx"][:, 24:NT])
                elif st_["nseg"] == 8:
                    nc.sync.dma_start(out_d[i], st_["ctx"][:])

            states = {}
            last = PER_CORE - 1
            ident_loaded = [False]
            # pair 0: strict phase order - the Act queue is in-order, so a
            # global act waiting on the global gather must not sit in front
            # of local acts whose data is already there
            states[0] = loads(0)
            nc.sync.dma_start(ident[:], ident_d[:])
            states[1] = loads(1)
            vt_init(states[0])
            exp_tiles(states[0])
            for p in range(4):
                gtok_pack(states[0], p)
            for p in range(8):
                local_pack(states[0], p, ms_dve=True)
                if p >= 4:
                    vt_pack(states[0], p - 4)   # local vt: kTL ready
            for p in range(8):
                global_pack(states[0], p,
                             engs={0: "v", 2: "v", 4: "v", 6: "v"})
                if p >= 4:
                    vt_pack(states[0], p)       # global vt: kTG ready
            vt_init(states[1])
            # middle pairs: interleaved with previous pair's PV
            for i in range(1, PER_CORE):
                if i == 1:
                    states[2] = loads(2)
                if i != 1:
                    vt_init(states[i])
                exp_tiles(states[i])
                # last pair: emit pack 7 first so the wrap PV segments can
                # start before the section ends
                rot = (7, 0, 1, 2, 3, 4, 5, 6) if i == last else tuple(range(8))
                for sl in range(8):
                    p = rot[sl]
                    if sl < 4:
                        gtok_pack(states[i], sl)
                    local_pack(states[i], p)
                    global_pack(states[i], p,
                                engs=GPACK_ENG if i != last else GPACK_ENG_LAST)
                    if sl < 2:
                        vt_pack(states[i], 2 * sl)
                        vt_pack(states[i], 2 * sl + 1)
                        if i == last:
                            vt_pack(states[i], 2 * sl + 4)
                            vt_pack(states[i], 2 * sl + 5)
                    elif i != last and sl >= 4:
                        vt_pack(states[i], sl)
                    pv_group(states[i - 1], i - 1,
                             (0, 1, 2, 3, 4, 5, 6, 7)[sl])
                    if i == last and sl >= 2:
                        pv_group(states[i], i, sl - 2)
            for s in (6, 7):
                pv_group(states[last], last, s)

    nc.compile()
    return nc


_CACHED = None


def _get_program():
    global _CACHED
    if _CACHED is None:
        _CACHED = build_program()
    return _CACHED


def _prep_core_inputs(q, k, v, gk, gv, lidx, gidx, pairs):
    """Build one core's input dict for its list of (n,h) pairs."""
    bf = ml_dtypes.bfloat16
    qTh = np.empty((PER_CORE, 64, QH_W), dtype=bf)
    kv = np.empty((PER_CORE, T, 128), dtype=bf)
    gkT = np.empty((PER_CORE, 64, 64), dtype=bf)
    gv1 = np.zeros((PER_CORE, 128, 2, 65), dtype=bf)
    li = np.empty((PER_CORE, 128, 256), dtype=np.int16)
    gi = np.empty((PER_CORE, 128, 256), dtype=np.int16)
    for s, (n, h) in enumerate(pairs):
        qt = np.ascontiguousarray(q[n, h].T)            # (64, T) f32
        qth = np.concatenate([qt[:, T - 128:], qt, qt[:, :256]], axis=1)
        qTh[s] = qth.astype(bf)
        kv[s, :, 0:64] = k[n, h].astype(bf)
        kv[s, :, 64:128] = v[n, h].astype(bf)
        gkT[s] = np.ascontiguousarray(gk[n, h].T).astype(bf)
        g1 = np.concatenate([gv[n, h], np.ones((64, 1), np.float32)],
                            axis=1).astype(bf)
        gv1[s, 0:64, 0] = g1      # parity 0: top half live
        gv1[s, 64:128, 1] = g1    # parity 1: bottom half live
        for arr, src in ((li, lidx), (gi, gidx)):
            ix = src[n, h, :, 0].astype(np.int16)       # (T,)
            arr[s] = np.tile(ix.reshape(T // 16, 16).T, (8, 1))
    ident = np.eye(128, dtype=bf)
    return {"qTh": qTh, "kvtab": kv, "gkT": gkT, "gv1": gv1,
            "lidx": li, "gidx": gi, "ident": ident}


def kernel(query_layer, key_layer, value_layer, attention_mask, local_idx,
           global_idx, global_key, global_value, global_mask):
    # attention_mask / global_mask are all-zero in this problem's input spec;
    # they contribute nothing to the scores and are not shipped to the device.
    q = np.asarray(query_layer, np.float32)
    k = np.asarray(key_layer, np.float32)
    v = np.asarray(value_layer, np.float32)
    gk = np.asarray(global_key, np.float32)
    gv = np.asarray(global_value, np.float32)
    li = np.asarray(local_idx)
    gi = np.asarray(global_idx)

    nc = _get_program()
    in_maps = []
    for m in range(NCORES):
        pairs = [((3 * m + s) // H, (3 * m + s) % H) for s in range(PER_CORE)]
        in_maps.append(_prep_core_inputs(q, k, v, gk, gv, li, gi, pairs))
    res = bass_utils.run_bass_kernel_spmd(nc, in_maps, core_ids=list(range(NCORES)))

    out = np.empty((N, H, T, D), np.float32)
    for m in range(NCORES):
        ctxT = np.asarray(res.results[m]["ctxT"]).astype(np.float32)
        tail = np.asarray(res.results[m]["ctxTail"]).astype(np.float32)
        ctxT[PER_CORE - 1, :, 24:] = tail
        for s in range(PER_CORE):
            n, h = (3 * m + s) // H, (3 * m + s) % H
            a = ctxT[s].transpose(1, 0, 2).reshape(T, 65)  # q-major rows
            out[n, h] = a[:, :64] / a[:, 64:65]
    return out
